# revision 44
# baseline (speedup 1.0000x reference)
"""AdditiveAttention TRN2 kernel v7 — sin-basis scores via low-u16 binade
phase extraction.

scores[q,k] = sum_h W_v[h] tanh(qh+kh) with tanh(s) ~= sum_m c_m sin(w_m s)
factorized through sin(a+b) = sin a cos b + cos a sin b into 2M rank-128
bf16 matmuls. Per-m pipeline:

  DVE  : p48_s = (w_m/8pi)*x + 48.0     f32 in binade [32,64): the low 16
  DVE  : p48_c = (w_m/8pi)*x + 48.0625  mantissa bits ARE the phase of
                                        w_m*x in 2^16 units (+1/16 value
                                        = +pi/2 phase for the cos row)
  ACT  : bas = Sin(lo16 * 2pi/2^16 - pi) -> bf16, reading the low u16 of
         each f32 via a strided bitcast view (no mask instructions)
  DVE  : qw  = bas_q * cw[m]            (per-partition scalar multiply)
  PE   : sc += qsw^T kc + qcw^T ks      (8x 512-col bf16 matmuls)

The base projections x = [W_k^T k^T | W_q^T q^T] are computed once in
bf16 (6 matmuls) and stay in PSUM; the per-m fmas read PSUM directly.
q (256) and k (1024) columns ride together in [128, 1280]-wide ops.
Emission is software-pipelined (fma pair of iteration m before qw/scores
of m-1) so no engine stalls behind a cross-engine dependency in-order.
Dummy Exp/Sin activations at t~0 pre-load both activation tables into
the two resident slots, keeping table loads off the critical path.
"""

import math

import ml_dtypes
import numpy as np

from concourse import bacc, mybir
from concourse import tile
from concourse.bass_utils import run_bass_kernel_spmd

B, LQ, LK, QS, KS, H, VS = 8, 256, 1024, 256, 256, 128, 256
F32 = mybir.dt.float32
BF16 = mybir.dt.bfloat16
FP8 = mybir.dt.float8e4
M_FP8 = 2  # terms >= this index use fp8 DoubleRow score matmuls

W_FIT = [0.29237, 0.87651, 1.51083, 2.50362]
C_FIT = [1.23737, 0.30825, 0.14462, 0.04779]
M = len(W_FIT)

SCALE_SIN = 2.0 * math.pi / (1 << 16)
NKC = LK // 128         # 8 key chunks of 128
W = LK + LQ             # 1280: k columns then q columns

_CACHE: dict = {}


def _build():
    nc = bacc.Bacc("TRN2", target_bir_lowering=False, debug=False)
    # packed params: fewer DMAs (each DMA costs ~620ns of queue issue time)
    qq = nc.declare_dram_parameter("qq", [128, 2, H + LQ], BF16, isOutput=False)
    kk = nc.declare_dram_parameter("kk", [128, 2, H + LK], BF16, isOutput=False)
    cn = nc.declare_dram_parameter("cn", [H, 3 + M], F32, isOutput=False)
    iv = nc.declare_dram_parameter("iv", [128, 1 + NKC, VS + 1], BF16,
                                   isOutput=False)
    # av rows: [sum_k attn*v | sum_k attn]; the division happens on host
    out = nc.declare_dram_parameter("out", [LQ, VS + 1], F32, isOutput=True)

    SIN = mybir.ActivationFunctionType.Sin
    EXP = mybir.ActivationFunctionType.Exp
    COPY = mybir.ActivationFunctionType.Copy
    ADD = mybir.AluOpType.add
    MULT = mybir.AluOpType.mult
    U16 = mybir.dt.uint16

    s_scale = [w / (8.0 * math.pi) for w in W_FIT]

    with tile.TileContext(nc) as tc:
        with (
            tc.tile_pool(name="const", bufs=1) as cpool,
            tc.tile_pool(name="p48p", bufs=3) as p48p,
            tc.tile_pool(name="bp", bufs=3) as bp,
            tc.tile_pool(name="qwp", bufs=3) as qwp,
            tc.tile_pool(name="ep", bufs=2) as ep,
            tc.tile_pool(name="etp", bufs=2) as etp,
            tc.tile_pool(name="ps_sc", bufs=1, space="PSUM") as ps_sc,
        ):
            qq_sb = cpool.tile([128, 2, H + LQ], BF16)
            kk_sb = cpool.tile([128, 2, H + LK], BF16)
            cn_sb = cpool.tile([128, 3 + M], F32)
            iv_sb = cpool.tile([128, 1 + NKC, VS + 1], BF16)
            base_sb = cpool.tile([128, 1, W], F32)
            bias2 = cpool.tile([128, 2, W], F32)
            dumm = cpool.tile([128, 1], F32)
            dumo = cpool.tile([128, 2], F32)
            wq_sb = qq_sb[:, :, 0:H]
            qTd_sb = qq_sb[:, :, H:H + LQ]
            wk_sb = kk_sb[:, :, 0:H]
            kTd_sb = kk_sb[:, :, H:H + LK]
            negpi_sb = cn_sb[:, 0:1]
            b48_sb = cn_sb[:, 1:2]
            b4806_sb = cn_sb[:, 2:3]
            cw_sb = cn_sb[:, 3:3 + M]
            ident_sb = iv_sb[:, 0, 0:128]
            vals_sb = iv_sb[:, 1:1 + NKC, :]

            # table pre-loads: Exp and Sin live in different act-func sets;
            # touching both on a dummy at t~0 pulls both tables into the two
            # resident slots while the DMAs run.
            nc.gpsimd.memset(dumm[:], 0.0)
            nc.gpsimd.memset(bias2[:, 0, :], 48.0)
            nc.gpsimd.memset(bias2[:, 1, :], 48.0625)
            nc.scalar.activation(dumo[:, 0:1], dumm[:], EXP)
            nc.scalar.activation(dumo[:, 1:2], dumm[:], SIN)

            # one transfer per engine DMA ring — rings serialize their own
            # transfers (~1KB/partition/us each), so spreading the inputs
            # across four rings lets all of them land in parallel.
            KA = H + 512
            nc.scalar.dma_start(out=qq_sb[:], in_=qq[:])
            nc.sync.dma_start(out=kk_sb[:, :, 0:KA], in_=kk[:, :, 0:KA])
            nc.gpsimd.dma_start(out=kk_sb[:, :, KA:], in_=kk[:, :, KA:])
            nc.gpsimd.dma_start(out=cn_sb[:], in_=cn[:])
            nc.gpsimd.dma_start(out=iv_sb[:], in_=iv[:])

            # sc[qb]: [128, 1024] f32 = 2 PSUM banks; matmuls write 512-col
            # halves (bank-aligned), exp reads the full 1024 in one call.
            sc = [ps_sc.tile([128, LK], F32, tag=f"sc{qb}", name=f"sc{qb}")
                  for qb in range(2)]

            with tc.tile_pool(name="ps_base", bufs=1, space="PSUM") as ps_base:
                # separate PSUM tiles per projection region: WAR tracking is
                # tile-granular, so a shared tile would serialize the k1
                # projections behind the DVE readers of k0/q.
                base_k0 = ps_base.tile([128, 1, 512], F32, tag="bk0")
                base_k1 = ps_base.tile([128, 1, 512], F32, tag="bk1")
                base_q = ps_base.tile([128, LQ], F32, tag="bq")

                def emit_fma(m, p48, lo, hi, src):
                    # one DVE op: (src*s_m) + [48.0 row | 48.0625 row]
                    nc.vector.scalar_tensor_tensor(
                        p48[:, :, lo:hi], src.to_broadcast([128, 2, hi - lo]),
                        float(s_scale[m]), bias2[:, :, lo:hi], MULT, ADD)

                def emit_sin(m, p48, bas, lo, hi):
                    nc.scalar.activation(
                        bas[:, :, lo:hi],
                        p48[:].bitcast(U16)[:, :, 2 * lo:2 * hi:2],
                        SIN, scale=SCALE_SIN, bias=negpi_sb)

                def emit_qw(m, bas):
                    # qw[:,0] = cw*sin_q pairs with cos_k; qw[:,1] = cw*cos_q
                    qw = qwp.tile([128, 2, LQ], BF16, tag="qw",
                                  name=f"qw_{m}")
                    nc.vector.tensor_scalar_mul(qw[:], bas[:, :, LK:W],
                                                cw_sb[:, m:m + 1])
                    return qw

                def emit_scores(m, bas, qw):
                    for qb in range(2):
                        for half in range(2):
                            nc.tensor.matmul(
                                sc[qb][:, 512 * half:512 * (half + 1)],
                                qw[:, 0, 128 * qb:128 * (qb + 1)],
                                bas[:, 1, 512 * half:512 * (half + 1)],
                                start=(m == 0), stop=False)
                            nc.tensor.matmul(
                                sc[qb][:, 512 * half:512 * (half + 1)],
                                qw[:, 1, 128 * qb:128 * (qb + 1)],
                                bas[:, 0, 512 * half:512 * (half + 1)],
                                start=False, stop=(m == M - 1))

                def emit_scores0_half(half, bas, qw):
                    for qb in range(2):
                        nc.tensor.matmul(
                            sc[qb][:, 512 * half:512 * (half + 1)],
                            qw[:, 0, 128 * qb:128 * (qb + 1)],
                            bas[:, 1, 512 * half:512 * (half + 1)],
                            start=True, stop=False)
                        nc.tensor.matmul(
                            sc[qb][:, 512 * half:512 * (half + 1)],
                            qw[:, 1, 128 * qb:128 * (qb + 1)],
                            bas[:, 0, 512 * half:512 * (half + 1)],
                            start=False, stop=False)

                # base = [khT | qhT]. PE order follows DMA arrival: k half 0,
                # then q, then k half 1. The m=0 fma/sin chain is split per
                # region so the first score matmuls only wait on the half-0
                # sin while half 1 is still in the DMA.
                p48_0 = p48p.tile([128, 2, W], F32, tag="p48", name="p48_0")
                bas_0 = bp.tile([128, 2, W], BF16, tag="bas", name="bas_0")
                for d in range(2):
                    nc.tensor.matmul(base_q[:], wq_sb[:, d, :],
                                     qTd_sb[:, d, :],
                                     start=(d == 0), stop=(d == 1))
                for d in range(2):
                    nc.tensor.matmul(base_k0[:, 0, :], wk_sb[:, d, :],
                                     kTd_sb[:, d, 0:512],
                                     start=(d == 0), stop=(d == 1))
                for d in range(2):
                    nc.tensor.matmul(base_k1[:, 0, :], wk_sb[:, d, :],
                                     kTd_sb[:, d, 512:1024],
                                     start=(d == 0), stop=(d == 1))
                # m=0 q chain entirely on ACT (Identity-fma, sin, Copy-mul):
                # no cross-engine handoffs, and DVE is free to run the k
                # chain the moment the k projections land.
                IDENT = mybir.ActivationFunctionType.Identity
                nc.scalar.activation(p48_0[:, 0, LK:W], base_q[:], IDENT,
                                     scale=float(s_scale[0]), bias=b48_sb)
                nc.scalar.activation(p48_0[:, 1, LK:W], base_q[:], IDENT,
                                     scale=float(s_scale[0]), bias=b4806_sb)
                emit_sin(0, p48_0, bas_0, LK, W)
                qw_0 = qwp.tile([128, 2, LQ], BF16, tag="qw", name="qw_0")
                nc.scalar.activation(qw_0[:], bas_0[:, :, LK:W], COPY,
                                     scale=cw_sb[:, 0:1])
                emit_fma(0, p48_0, 0, 512, base_k0[:])
                emit_sin(0, p48_0, bas_0, 0, 512)
                emit_fma(0, p48_0, 512, 1024, base_k1[:])
                emit_sin(0, p48_0, bas_0, 512, 1024)
                emit_scores0_half(0, bas_0, qw_0)
                # PSUM->SBUF base copy in DVE slack: DVE reads PSUM at half
                # throughput, so the 2(M-1) steady-state fmas read SBUF.
                nc.vector.tensor_copy(base_sb[:, 0, 0:512], base_k0[:, 0, :])
                nc.vector.tensor_copy(base_sb[:, 0, 512:1024],
                                      base_k1[:, 0, :])
                nc.vector.tensor_copy(base_sb[:, 0, LK:W], base_q[:])
                emit_scores0_half(1, bas_0, qw_0)

                def emit_qw8(m, bas8):
                    # DoubleRow pairs lhsT row i with rhs row i; the q rows
                    # are crossed so row0 = cw*cos_q meets sin_k and
                    # row1 = cw*sin_q meets cos_k.
                    qw8 = qwp.tile([128, 2, LQ], FP8, tag="qw8",
                                   name=f"qw8_{m}")
                    nc.vector.tensor_scalar_mul(qw8[:, 0, :],
                                                bas8[:, 1, LK:W],
                                                cw_sb[:, m:m + 1])
                    nc.vector.tensor_scalar_mul(qw8[:, 1, :],
                                                bas8[:, 0, LK:W],
                                                cw_sb[:, m:m + 1])
                    return qw8

                def emit_scores8(m, bas8, qw8):
                    for qb in range(2):
                        for half in range(2):
                            nc.tensor.matmul(
                                sc[qb][:, 512 * half:512 * (half + 1)],
                                qw8[:, :, 128 * qb:128 * (qb + 1)],
                                bas8[:, :, 512 * half:512 * (half + 1)],
                                start=False, stop=(m == M - 1),
                                perf_mode=mybir.MatmulPerfMode.DoubleRow)

                # software-pipelined: on the in-order DVE queue, the fma pair
                # of iteration m+1 is emitted before qw_m (which waits on the
                # ACT sin), so the fma feeding sin_{m+1} is never stuck
                # behind a cross-engine dependency. Terms m >= M_FP8 run the
                # score matmuls in fp8 DoubleRow (one matmul per sc half).
                prev, prev_qw = bas_0, qw_0
                prev_m = 0
                for m in range(1, M):
                    p48 = p48p.tile([128, 2, W], F32, tag="p48",
                                    name=f"p48_{m}")
                    emit_fma(m, p48, 0, W, base_sb[:])
                    if prev_m >= 1:
                        prev_qw = (emit_qw8(prev_m, prev)
                                   if prev_m >= M_FP8 else
                                   emit_qw(prev_m, prev))
                    if prev_m >= M_FP8:
                        emit_scores8(prev_m, prev, prev_qw)
                    else:
                        emit_scores(prev_m, prev, prev_qw)
                    bas = bp.tile([128, 2, W],
                                  FP8 if m >= M_FP8 else BF16,
                                  tag="bas8" if m >= M_FP8 else "bas",
                                  name=f"bas_{m}")
                    emit_sin(m, p48, bas, 0, W)
                    prev, prev_m = bas, m
                prev_qw = (emit_qw8(M - 1, prev) if M - 1 >= M_FP8
                           else emit_qw(M - 1, prev))
                if M - 1 >= M_FP8:
                    emit_scores8(M - 1, prev, prev_qw)
                else:
                    emit_scores(M - 1, prev, prev_qw)

            with tc.tile_pool(name="ps_tail", bufs=2, space="PSUM") as ps_tail:
                expS = [None, None]
                for qb in range(2):
                    expS[qb] = ep.tile([128, LK], BF16, tag="exps",
                                       name=f"expS{qb}")
                    nc.scalar.activation(expS[qb][:], sc[qb][:], EXP,
                                         scale=1.0 / 16.0)
                for qb in range(2):
                    # interleave transpose and AV accumulation: av_c runs as
                    # soon as chunk c's PSUM->SBUF copy lands, so the last
                    # AV matmul trails the last transpose by one chunk.
                    expT = etp.tile([128, NKC, 128], BF16, tag="expt")
                    av = ps_tail.tile([128, VS + 1], F32, tag="av",
                                      name=f"av{qb}")

                    def tpc(c, qb=qb, expT=expT):
                        tp = ps_tail.tile([128, 128], BF16, tag="tp",
                                          name=f"tp{qb}{c}")
                        nc.tensor.transpose(tp[:],
                                            expS[qb][:, 128 * c:128 * (c + 1)],
                                            ident_sb[:])
                        nc.vector.tensor_copy(expT[:, c, :], tp[:])

                    tpc(0)
                    tpc(1)
                    for c in range(NKC):
                        if c + 2 < NKC:
                            tpc(c + 2)
                        nc.tensor.matmul(av[:], expT[:, c, :], vals_sb[:, c, :],
                                         start=(c == 0), stop=(c == NKC - 1))
                    # numerator and denominator ship together; host divides.
                    o_sb = ep.tile([128, VS + 1], F32, tag="osb",
                                   name=f"osb{qb}")
                    if qb == 0:
                        nc.vector.tensor_copy(o_sb[:], av[:])
                        nc.sync.dma_start(out=out[0:128, :], in_=o_sb[:])
                    else:
                        nc.scalar.activation(o_sb[:], av[:], COPY)
                        nc.gpsimd.dma_start(out=out[128:256, :], in_=o_sb[:])

    nc.compile()
    return nc


def _pack_rows(a):
    # [256, N] -> [128, 2, N]: row r -> (r % 128, r // 128)
    return np.ascontiguousarray(a.reshape(2, 128, -1).transpose(1, 0, 2))


def _make_in_maps(inputs) -> list[dict]:
    queries = np.asarray(inputs["queries"], dtype=np.float32)
    key = np.asarray(inputs["key"], dtype=np.float32)
    value = np.asarray(inputs["value"], dtype=np.float32)
    vl = np.asarray(inputs["valid_length"], dtype=np.int32)
    W_q = np.asarray(inputs["W_q"], dtype=np.float32)
    W_k = np.asarray(inputs["W_k"], dtype=np.float32)
    W_v = np.asarray(inputs["W_v"], dtype=np.float32)

    cfit = np.asarray(C_FIT, np.float32)
    cn = np.empty((H, 3 + M), np.float32)
    cn[:, 0] = -math.pi
    cn[:, 1] = 48.0
    cn[:, 2] = 48.0625
    cn[:, 3:] = 16.0 * W_v[:, None] * cfit[None, :]
    cn = np.ascontiguousarray(cn)
    wk_b = _pack_rows(W_k.astype(ml_dtypes.bfloat16))
    wq_b = _pack_rows(W_q.astype(ml_dtypes.bfloat16))

    in_maps = []
    for b in range(B):
        v = max(int(vl[b]), 0)
        vals = np.zeros((LK, VS + 1), dtype=np.float32)
        vals[:v, :VS] = value[b, :v]
        vals[:v, VS] = 1.0
        iv = np.zeros((128, 1 + NKC, VS + 1), dtype=ml_dtypes.bfloat16)
        iv[:, 0, 0:128] = np.eye(128, dtype=ml_dtypes.bfloat16)
        iv[:, 1:, :] = vals.astype(ml_dtypes.bfloat16).reshape(
            NKC, 128, VS + 1).transpose(1, 0, 2)
        qq = np.concatenate(
            [wq_b, _pack_rows(queries[b].T.astype(ml_dtypes.bfloat16))],
            axis=2)
        kk = np.concatenate(
            [wk_b, _pack_rows(key[b].T.astype(ml_dtypes.bfloat16))],
            axis=2)
        in_maps.append({
            "qq": np.ascontiguousarray(qq),
            "kk": np.ascontiguousarray(kk),
            "cn": cn, "iv": np.ascontiguousarray(iv),
        })
    return in_maps


def _postprocess(res, inputs) -> np.ndarray:
    value = np.asarray(inputs["value"], dtype=np.float32)
    vl = np.asarray(inputs["valid_length"], dtype=np.int32)
    av = np.stack([np.asarray(res.results[i]["out"]) for i in range(B)], axis=0)
    with np.errstate(divide="ignore", invalid="ignore"):
        out = av[:, :, :VS] / av[:, :, VS:VS + 1]
    for b in range(B):
        if int(vl[b]) <= 0:
            out[b] = value[b].mean(axis=0, keepdims=True)
    return out.astype(np.float32)


def _sane(out, inputs) -> bool:
    # each output row is a convex combination of value rows, so it must be
    # finite and lie within the per-batch value range; a corrupted run
    # (transient device glitch) violates this with near-certainty.
    if not np.isfinite(out).all():
        return False
    value = np.asarray(inputs["value"], dtype=np.float32)
    bound = np.abs(value).max(axis=(1, 2)) * 1.05 + 0.1
    return bool((np.abs(out).max(axis=(1, 2)) <= bound).all())


def kernel(**inputs) -> np.ndarray:
    if "nc" not in _CACHE:
        _CACHE["nc"] = _build()
    nc = _CACHE["nc"]
    in_maps = _make_in_maps(inputs)
    for _attempt in range(3):
        res = run_bass_kernel_spmd(nc, in_maps, core_ids=list(range(B)))
        out = _postprocess(res, inputs)
        if _sane(out, inputs):
            break
    return out


# revision 45
# speedup vs baseline: 1.0508x; 1.0508x over previous
"""AdditiveAttention TRN2 kernel v7 — sin-basis scores via low-u16 binade
phase extraction.

scores[q,k] = sum_h W_v[h] tanh(qh+kh) with tanh(s) ~= sum_m c_m sin(w_m s)
factorized through sin(a+b) = sin a cos b + cos a sin b into 2M rank-128
bf16 matmuls. Per-m pipeline:

  DVE  : p48_s = (w_m/8pi)*x + 48.0     f32 in binade [32,64): the low 16
  DVE  : p48_c = (w_m/8pi)*x + 48.0625  mantissa bits ARE the phase of
                                        w_m*x in 2^16 units (+1/16 value
                                        = +pi/2 phase for the cos row)
  ACT  : bas = Sin(lo16 * 2pi/2^16 - pi) -> bf16, reading the low u16 of
         each f32 via a strided bitcast view (no mask instructions)
  DVE  : qw  = bas_q * cw[m]            (per-partition scalar multiply)
  PE   : sc += qsw^T kc + qcw^T ks      (8x 512-col bf16 matmuls)

The base projections x = [W_k^T k^T | W_q^T q^T] are computed once in
bf16 (6 matmuls) and stay in PSUM; the per-m fmas read PSUM directly.
q (256) and k (1024) columns ride together in [128, 1280]-wide ops.
Emission is software-pipelined (fma pair of iteration m before qw/scores
of m-1) so no engine stalls behind a cross-engine dependency in-order.
Dummy Exp/Sin activations at t~0 pre-load both activation tables into
the two resident slots, keeping table loads off the critical path.
"""

import math

import ml_dtypes
import numpy as np

from concourse import bacc, mybir
from concourse import tile
from concourse.bass_utils import run_bass_kernel_spmd

B, LQ, LK, QS, KS, H, VS = 8, 256, 1024, 256, 256, 128, 256
F32 = mybir.dt.float32
BF16 = mybir.dt.bfloat16
FP8 = mybir.dt.float8e4
M_FP8 = 2  # terms >= this index use fp8 DoubleRow score matmuls

W_FIT = [0.29237, 0.87651, 1.51083, 2.50362]
C_FIT = [1.23737, 0.30825, 0.14462, 0.04779]
M = len(W_FIT)

SCALE_SIN = 2.0 * math.pi / (1 << 16)
NKC = LK // 128         # 8 key chunks of 128
W = LK + LQ             # 1280: k columns then q columns

_CACHE: dict = {}


def _build():
    nc = bacc.Bacc("TRN2", target_bir_lowering=False, debug=False)
    # packed params: fewer DMAs (each DMA costs ~620ns of queue issue time)
    qq = nc.declare_dram_parameter("qq", [128, 2, H + LQ], BF16, isOutput=False)
    kk = nc.declare_dram_parameter("kk", [128, 2, H + LK], BF16, isOutput=False)
    cn = nc.declare_dram_parameter("cn", [H, 3 + M], F32, isOutput=False)
    iv = nc.declare_dram_parameter("iv", [128, 1 + NKC, VS + 1], BF16,
                                   isOutput=False)
    # av rows: [sum_k attn*v | sum_k attn]; the division happens on host
    out = nc.declare_dram_parameter("out", [LQ, VS + 1], F32, isOutput=True)

    SIN = mybir.ActivationFunctionType.Sin
    EXP = mybir.ActivationFunctionType.Exp
    COPY = mybir.ActivationFunctionType.Copy
    ADD = mybir.AluOpType.add
    MULT = mybir.AluOpType.mult
    U16 = mybir.dt.uint16

    s_scale = [w / (8.0 * math.pi) for w in W_FIT]

    with tile.TileContext(nc) as tc:
        with (
            tc.tile_pool(name="const", bufs=1) as cpool,
            tc.tile_pool(name="p48p", bufs=3) as p48p,
            tc.tile_pool(name="bp", bufs=3) as bp,
            tc.tile_pool(name="qwp", bufs=3) as qwp,
            tc.tile_pool(name="ep", bufs=2) as ep,
            tc.tile_pool(name="etp", bufs=2) as etp,
            tc.tile_pool(name="ps_sc", bufs=1, space="PSUM") as ps_sc,
        ):
            qq_sb = cpool.tile([128, 2, H + LQ], BF16)
            kk_sb = cpool.tile([128, 2, H + LK], BF16)
            cn_sb = cpool.tile([128, 3 + M], F32)
            iv_sb = cpool.tile([128, 1 + NKC, VS + 1], BF16)
            base_sb = cpool.tile([128, W], F32)
            dumm = cpool.tile([128, 1], F32)
            dumo = cpool.tile([128, 2], F32)
            wq_sb = qq_sb[:, :, 0:H]
            qTd_sb = qq_sb[:, :, H:H + LQ]
            wk_sb = kk_sb[:, :, 0:H]
            kTd_sb = kk_sb[:, :, H:H + LK]
            negpi_sb = cn_sb[:, 0:1]
            b48_sb = cn_sb[:, 1:2]
            b4806_sb = cn_sb[:, 2:3]
            cw_sb = cn_sb[:, 3:3 + M]
            ident_sb = iv_sb[:, 0, 0:128]
            vals_sb = iv_sb[:, 1:1 + NKC, :]

            # table pre-loads: Exp and Sin live in different act-func sets;
            # touching both on a dummy at t~0 pulls both tables into the two
            # resident slots while the DMAs run.
            nc.gpsimd.memset(dumm[:], 0.0)
            nc.scalar.activation(dumo[:, 0:1], dumm[:], EXP)
            nc.scalar.activation(dumo[:, 1:2], dumm[:], SIN)

            # one transfer per engine DMA ring — rings serialize their own
            # transfers (~1KB/partition/us each), so spreading the inputs
            # across four rings lets all of them land in parallel.
            KA = H + 512
            nc.scalar.dma_start(out=qq_sb[:], in_=qq[:])
            nc.sync.dma_start(out=kk_sb[:, :, 0:KA], in_=kk[:, :, 0:KA])
            nc.gpsimd.dma_start(out=kk_sb[:, :, KA:], in_=kk[:, :, KA:])
            nc.gpsimd.dma_start(out=cn_sb[:], in_=cn[:])
            nc.gpsimd.dma_start(out=iv_sb[:], in_=iv[:])

            # sc[qb]: [128, 1024] f32 = 2 PSUM banks; matmuls write 512-col
            # halves (bank-aligned), exp reads the full 1024 in one call.
            sc = [ps_sc.tile([128, LK], F32, tag=f"sc{qb}", name=f"sc{qb}")
                  for qb in range(2)]

            with tc.tile_pool(name="ps_base", bufs=1, space="PSUM") as ps_base:
                # separate PSUM tiles per projection region: WAR tracking is
                # tile-granular, so a shared tile would serialize the k1
                # projections behind the DVE readers of k0/q.
                base_k0 = ps_base.tile([128, 512], F32, tag="bk0")
                base_k1 = ps_base.tile([128, 512], F32, tag="bk1")
                base_q = ps_base.tile([128, LQ], F32, tag="bq")

                def emit_fma(m, p48, lo, hi, src):
                    nc.vector.tensor_scalar(p48[:, 0, lo:hi], src,
                                            float(s_scale[m]), 48.0,
                                            MULT, ADD)
                    nc.vector.tensor_scalar(p48[:, 1, lo:hi], src,
                                            float(s_scale[m]), 48.0625,
                                            MULT, ADD)

                def emit_sin(m, p48, bas, lo, hi):
                    nc.scalar.activation(
                        bas[:, :, lo:hi],
                        p48[:].bitcast(U16)[:, :, 2 * lo:2 * hi:2],
                        SIN, scale=SCALE_SIN, bias=negpi_sb)

                def emit_qw(m, bas):
                    # qw[:,0] = cw*sin_q pairs with cos_k; qw[:,1] = cw*cos_q
                    qw = qwp.tile([128, 2, LQ], BF16, tag="qw",
                                  name=f"qw_{m}")
                    nc.vector.tensor_scalar_mul(qw[:], bas[:, :, LK:W],
                                                cw_sb[:, m:m + 1])
                    return qw

                def emit_scores(m, bas, qw):
                    for qb in range(2):
                        for half in range(2):
                            nc.tensor.matmul(
                                sc[qb][:, 512 * half:512 * (half + 1)],
                                qw[:, 0, 128 * qb:128 * (qb + 1)],
                                bas[:, 1, 512 * half:512 * (half + 1)],
                                start=(m == 0), stop=False)
                            nc.tensor.matmul(
                                sc[qb][:, 512 * half:512 * (half + 1)],
                                qw[:, 1, 128 * qb:128 * (qb + 1)],
                                bas[:, 0, 512 * half:512 * (half + 1)],
                                start=False, stop=(m == M - 1))

                def emit_scores0_half(half, bas, qw):
                    for qb in range(2):
                        nc.tensor.matmul(
                            sc[qb][:, 512 * half:512 * (half + 1)],
                            qw[:, 0, 128 * qb:128 * (qb + 1)],
                            bas[:, 1, 512 * half:512 * (half + 1)],
                            start=True, stop=False)
                        nc.tensor.matmul(
                            sc[qb][:, 512 * half:512 * (half + 1)],
                            qw[:, 1, 128 * qb:128 * (qb + 1)],
                            bas[:, 0, 512 * half:512 * (half + 1)],
                            start=False, stop=False)

                # base = [khT | qhT]. PE order follows DMA arrival: k half 0,
                # then q, then k half 1. The m=0 fma/sin chain is split per
                # region so the first score matmuls only wait on the half-0
                # sin while half 1 is still in the DMA.
                p48_0 = p48p.tile([128, 2, W], F32, tag="p48", name="p48_0")
                bas_0 = bp.tile([128, 2, W], BF16, tag="bas", name="bas_0")
                for d in range(2):
                    nc.tensor.matmul(base_q[:], wq_sb[:, d, :],
                                     qTd_sb[:, d, :],
                                     start=(d == 0), stop=(d == 1))
                for d in range(2):
                    nc.tensor.matmul(base_k0[:], wk_sb[:, d, :],
                                     kTd_sb[:, d, 0:512],
                                     start=(d == 0), stop=(d == 1))
                for d in range(2):
                    nc.tensor.matmul(base_k1[:], wk_sb[:, d, :],
                                     kTd_sb[:, d, 512:1024],
                                     start=(d == 0), stop=(d == 1))
                # m=0 q chain entirely on ACT (Identity-fma, sin, Copy-mul):
                # no cross-engine handoffs, and DVE is free to run the k
                # chain the moment the k projections land.
                IDENT = mybir.ActivationFunctionType.Identity
                nc.scalar.activation(p48_0[:, 0, LK:W], base_q[:], IDENT,
                                     scale=float(s_scale[0]), bias=b48_sb)
                nc.scalar.activation(p48_0[:, 1, LK:W], base_q[:], IDENT,
                                     scale=float(s_scale[0]), bias=b4806_sb)
                emit_sin(0, p48_0, bas_0, LK, W)
                qw_0 = qwp.tile([128, 2, LQ], BF16, tag="qw", name="qw_0")
                nc.scalar.activation(qw_0[:], bas_0[:, :, LK:W], COPY,
                                     scale=cw_sb[:, 0:1])
                emit_fma(0, p48_0, 0, 512, base_k0[:])
                emit_sin(0, p48_0, bas_0, 0, 512)
                emit_fma(0, p48_0, 512, 1024, base_k1[:])
                emit_sin(0, p48_0, bas_0, 512, 1024)
                emit_scores0_half(0, bas_0, qw_0)
                # PSUM->SBUF base copy in DVE slack: DVE reads PSUM at half
                # throughput, so the 2(M-1) steady-state fmas read SBUF.
                nc.vector.tensor_copy(base_sb[:, 0:512], base_k0[:])
                nc.vector.tensor_copy(base_sb[:, 512:1024], base_k1[:])
                nc.vector.tensor_copy(base_sb[:, LK:W], base_q[:])
                emit_scores0_half(1, bas_0, qw_0)

                def emit_qw8(m, bas8):
                    # DoubleRow pairs lhsT row i with rhs row i; the q rows
                    # are crossed so row0 = cw*cos_q meets sin_k and
                    # row1 = cw*sin_q meets cos_k.
                    qw8 = qwp.tile([128, 2, LQ], FP8, tag="qw8",
                                   name=f"qw8_{m}")
                    nc.vector.tensor_scalar_mul(qw8[:, 0, :],
                                                bas8[:, 1, LK:W],
                                                cw_sb[:, m:m + 1])
                    nc.vector.tensor_scalar_mul(qw8[:, 1, :],
                                                bas8[:, 0, LK:W],
                                                cw_sb[:, m:m + 1])
                    return qw8

                def emit_scores8(m, bas8, qw8):
                    for qb in range(2):
                        for half in range(2):
                            nc.tensor.matmul(
                                sc[qb][:, 512 * half:512 * (half + 1)],
                                qw8[:, :, 128 * qb:128 * (qb + 1)],
                                bas8[:, :, 512 * half:512 * (half + 1)],
                                start=False, stop=(m == M - 1),
                                perf_mode=mybir.MatmulPerfMode.DoubleRow)

                # software-pipelined: on the in-order DVE queue, the fma pair
                # of iteration m+1 is emitted before qw_m (which waits on the
                # ACT sin), so the fma feeding sin_{m+1} is never stuck
                # behind a cross-engine dependency. Terms m >= M_FP8 run the
                # score matmuls in fp8 DoubleRow (one matmul per sc half).
                prev, prev_qw = bas_0, qw_0
                prev_m = 0
                for m in range(1, M):
                    p48 = p48p.tile([128, 2, W], F32, tag="p48",
                                    name=f"p48_{m}")
                    emit_fma(m, p48, 0, W, base_sb[:])
                    if prev_m >= 1:
                        prev_qw = (emit_qw8(prev_m, prev)
                                   if prev_m >= M_FP8 else
                                   emit_qw(prev_m, prev))
                    if prev_m >= M_FP8:
                        emit_scores8(prev_m, prev, prev_qw)
                    else:
                        emit_scores(prev_m, prev, prev_qw)
                    bas = bp.tile([128, 2, W],
                                  FP8 if m >= M_FP8 else BF16,
                                  tag="bas8" if m >= M_FP8 else "bas",
                                  name=f"bas_{m}")
                    emit_sin(m, p48, bas, 0, W)
                    prev, prev_m = bas, m
                prev_qw = (emit_qw8(M - 1, prev) if M - 1 >= M_FP8
                           else emit_qw(M - 1, prev))
                if M - 1 >= M_FP8:
                    emit_scores8(M - 1, prev, prev_qw)
                else:
                    emit_scores(M - 1, prev, prev_qw)

            with tc.tile_pool(name="ps_tail", bufs=2, space="PSUM") as ps_tail:
                expS = [None, None]
                for qb in range(2):
                    expS[qb] = ep.tile([128, LK], BF16, tag="exps",
                                       name=f"expS{qb}")
                    nc.scalar.activation(expS[qb][:], sc[qb][:], EXP,
                                         scale=1.0 / 16.0)
                for qb in range(2):
                    # interleave transpose and AV accumulation: av_c runs as
                    # soon as chunk c's PSUM->SBUF copy lands, so the last
                    # AV matmul trails the last transpose by one chunk.
                    expT = etp.tile([128, NKC, 128], BF16, tag="expt")
                    av = ps_tail.tile([128, VS + 1], F32, tag="av",
                                      name=f"av{qb}")

                    def tpc(c, qb=qb, expT=expT):
                        tp = ps_tail.tile([128, 128], BF16, tag="tp",
                                          name=f"tp{qb}{c}")
                        nc.tensor.transpose(tp[:],
                                            expS[qb][:, 128 * c:128 * (c + 1)],
                                            ident_sb[:])
                        nc.vector.tensor_copy(expT[:, c, :], tp[:])

                    tpc(0)
                    tpc(1)
                    for c in range(NKC):
                        if c + 2 < NKC:
                            tpc(c + 2)
                        nc.tensor.matmul(av[:], expT[:, c, :], vals_sb[:, c, :],
                                         start=(c == 0), stop=(c == NKC - 1))
                    # numerator and denominator ship together; host divides.
                    o_sb = ep.tile([128, VS + 1], F32, tag="osb",
                                   name=f"osb{qb}")
                    if qb == 0:
                        nc.vector.tensor_copy(o_sb[:], av[:])
                        nc.sync.dma_start(out=out[0:128, :], in_=o_sb[:])
                    else:
                        nc.scalar.activation(o_sb[:], av[:], COPY)
                        nc.gpsimd.dma_start(out=out[128:256, :], in_=o_sb[:])

    nc.compile()
    return nc


def _pack_rows(a):
    # [256, N] -> [128, 2, N]: row r -> (r % 128, r // 128)
    return np.ascontiguousarray(a.reshape(2, 128, -1).transpose(1, 0, 2))


def _make_in_maps(inputs) -> list[dict]:
    queries = np.asarray(inputs["queries"], dtype=np.float32)
    key = np.asarray(inputs["key"], dtype=np.float32)
    value = np.asarray(inputs["value"], dtype=np.float32)
    vl = np.asarray(inputs["valid_length"], dtype=np.int32)
    W_q = np.asarray(inputs["W_q"], dtype=np.float32)
    W_k = np.asarray(inputs["W_k"], dtype=np.float32)
    W_v = np.asarray(inputs["W_v"], dtype=np.float32)

    cfit = np.asarray(C_FIT, np.float32)
    cn = np.empty((H, 3 + M), np.float32)
    cn[:, 0] = -math.pi
    cn[:, 1] = 48.0
    cn[:, 2] = 48.0625
    cn[:, 3:] = 16.0 * W_v[:, None] * cfit[None, :]
    cn = np.ascontiguousarray(cn)
    wk_b = _pack_rows(W_k.astype(ml_dtypes.bfloat16))
    wq_b = _pack_rows(W_q.astype(ml_dtypes.bfloat16))

    in_maps = []
    for b in range(B):
        v = max(int(vl[b]), 0)
        vals = np.zeros((LK, VS + 1), dtype=np.float32)
        vals[:v, :VS] = value[b, :v]
        vals[:v, VS] = 1.0
        iv = np.zeros((128, 1 + NKC, VS + 1), dtype=ml_dtypes.bfloat16)
        iv[:, 0, 0:128] = np.eye(128, dtype=ml_dtypes.bfloat16)
        iv[:, 1:, :] = vals.astype(ml_dtypes.bfloat16).reshape(
            NKC, 128, VS + 1).transpose(1, 0, 2)
        qq = np.concatenate(
            [wq_b, _pack_rows(queries[b].T.astype(ml_dtypes.bfloat16))],
            axis=2)
        kk = np.concatenate(
            [wk_b, _pack_rows(key[b].T.astype(ml_dtypes.bfloat16))],
            axis=2)
        in_maps.append({
            "qq": np.ascontiguousarray(qq),
            "kk": np.ascontiguousarray(kk),
            "cn": cn, "iv": np.ascontiguousarray(iv),
        })
    return in_maps


def _postprocess(res, inputs) -> np.ndarray:
    value = np.asarray(inputs["value"], dtype=np.float32)
    vl = np.asarray(inputs["valid_length"], dtype=np.int32)
    av = np.stack([np.asarray(res.results[i]["out"]) for i in range(B)], axis=0)
    with np.errstate(divide="ignore", invalid="ignore"):
        out = av[:, :, :VS] / av[:, :, VS:VS + 1]
    for b in range(B):
        if int(vl[b]) <= 0:
            out[b] = value[b].mean(axis=0, keepdims=True)
    return out.astype(np.float32)


def _sane(out, inputs) -> bool:
    # each output row is a convex combination of value rows, so it must be
    # finite and lie within the per-batch value range; a corrupted run
    # (transient device glitch) violates this with near-certainty.
    if not np.isfinite(out).all():
        return False
    value = np.asarray(inputs["value"], dtype=np.float32)
    bound = np.abs(value).max(axis=(1, 2)) * 1.05 + 0.1
    return bool((np.abs(out).max(axis=(1, 2)) <= bound).all())


def kernel(**inputs) -> np.ndarray:
    if "nc" not in _CACHE:
        _CACHE["nc"] = _build()
    nc = _CACHE["nc"]
    in_maps = _make_in_maps(inputs)
    for _attempt in range(3):
        res = run_bass_kernel_spmd(nc, in_maps, core_ids=list(range(B)))
        out = _postprocess(res, inputs)
        if _sane(out, inputs):
            break
    return out


# revision 46
# speedup vs baseline: 1.0616x; 1.0102x over previous
"""AdditiveAttention TRN2 kernel v8 — sin-basis scores via low-u16 binade
phase extraction, M=4 terms, fp8 DoubleRow for the small-coefficient terms.

scores[q,k] = sum_h W_v[h] tanh(qh+kh) with tanh(s) ~= sum_m c_m sin(w_m s)
factorized through sin(a+b) = sin a cos b + cos a sin b into 2M rank-128
matmuls per core (data-parallel over batch, one batch per NeuronCore).
Per-m pipeline:

  DVE  : p48_s = (w_m/8pi)*x + 48.0     f32 in binade [32,64): the low 16
  DVE  : p48_c = (w_m/8pi)*x + 48.0625  mantissa bits ARE the phase of
                                        w_m*x in 2^16 units (+1/16 value
                                        = +pi/2 phase for the cos row)
  ACT  : bas = Sin(lo16 * 2pi/2^16 - pi), reading the low u16 of each f32
         via a strided bitcast view — no range-reduction mask instructions
  DVE  : qw  = bas_q * (16*cw[m])       (per-partition scalar multiply)
  PE   : sc += qsw^T kc + qcw^T ks      m < 2: 8x 512-col bf16 matmuls
                                        m >= 2: 4x fp8e4 DoubleRow matmuls
                                        (both trig products in one pass)

The 16x weight scale keeps fp8 magnitudes healthy and is undone by the
Exp activation's scale=1/16. The base projections [khT | qhT] run once in
bf16 into three separate PSUM tiles (separate tiles because WAR tracking
is tile-granular), then copy to SBUF where DVE reads at 2 elem/cycle
(PSUM reads run at half rate). q (256) and k (1024) columns ride together
in [128, 1280]-wide ops. m=0 is split per region (q / k-half0 / k-half1)
with its q chain entirely on ACT, so the first score matmuls launch while
the second half of the k DMA is still in flight. Emission is
software-pipelined: the fma pair of iteration m+1 is emitted before qw_m
on the in-order DVE queue. Inputs arrive as three packed DMAs spread
across the SP/ACT/Pool rings; dummy Exp/Sin activations at t~0 pre-load
both activation tables. The softmax tail transposes exp(sc) via the PE,
interleaving transposes with the attn@V accumulation; numerator and
denominator ship to HBM together and the host divides.
"""

import math

import ml_dtypes
import numpy as np

from concourse import bacc, mybir
from concourse import tile
from concourse.bass_utils import run_bass_kernel_spmd

B, LQ, LK, QS, KS, H, VS = 8, 256, 1024, 256, 256, 128, 256
F32 = mybir.dt.float32
BF16 = mybir.dt.bfloat16
FP8 = mybir.dt.float8e4
M_FP8 = 2  # terms >= this index use fp8 DoubleRow score matmuls

W_FIT = [0.29237, 0.87651, 1.51083, 2.50362]
C_FIT = [1.23737, 0.30825, 0.14462, 0.04779]
M = len(W_FIT)

SCALE_SIN = 2.0 * math.pi / (1 << 16)
NKC = LK // 128         # 8 key chunks of 128
W = LK + LQ             # 1280: k columns then q columns

_CACHE: dict = {}


def _build():
    nc = bacc.Bacc("TRN2", target_bir_lowering=False, debug=False)
    # packed params: fewer DMAs (each DMA costs ~620ns of queue issue time)
    qq = nc.declare_dram_parameter("qq", [128, 2, H + LQ], BF16, isOutput=False)
    kk = nc.declare_dram_parameter("kk", [128, 2, H + LK], BF16, isOutput=False)
    cn = nc.declare_dram_parameter("cn", [H, 3 + M], F32, isOutput=False)
    iv = nc.declare_dram_parameter("iv", [128, 1 + NKC, VS + 1], BF16,
                                   isOutput=False)
    # av rows: [sum_k attn*v | sum_k attn]; the division happens on host
    out = nc.declare_dram_parameter("out", [LQ, VS + 1], F32, isOutput=True)

    SIN = mybir.ActivationFunctionType.Sin
    EXP = mybir.ActivationFunctionType.Exp
    COPY = mybir.ActivationFunctionType.Copy
    ADD = mybir.AluOpType.add
    MULT = mybir.AluOpType.mult
    U16 = mybir.dt.uint16

    s_scale = [w / (8.0 * math.pi) for w in W_FIT]

    with tile.TileContext(nc) as tc:
        with (
            tc.tile_pool(name="const", bufs=1) as cpool,
            tc.tile_pool(name="p48p", bufs=3) as p48p,
            tc.tile_pool(name="bp", bufs=3) as bp,
            tc.tile_pool(name="qwp", bufs=3) as qwp,
            tc.tile_pool(name="ep", bufs=2) as ep,
            tc.tile_pool(name="etp", bufs=2) as etp,
            tc.tile_pool(name="ps_sc", bufs=1, space="PSUM") as ps_sc,
        ):
            qq_sb = cpool.tile([128, 2, H + LQ], BF16)
            kk_sb = cpool.tile([128, 2, H + LK], BF16)
            cn_sb = cpool.tile([128, 3 + M], F32)
            iv_sb = cpool.tile([128, 1 + NKC, VS + 1], BF16)
            base_sb = cpool.tile([128, W], F32)
            dumm = cpool.tile([128, 1], F32)
            dumo = cpool.tile([128, 2], F32)
            wq_sb = qq_sb[:, :, 0:H]
            qTd_sb = qq_sb[:, :, H:H + LQ]
            wk_sb = kk_sb[:, :, 0:H]
            kTd_sb = kk_sb[:, :, H:H + LK]
            negpi_sb = cn_sb[:, 0:1]
            b48_sb = cn_sb[:, 1:2]
            b4806_sb = cn_sb[:, 2:3]
            cw_sb = cn_sb[:, 3:3 + M]
            ident_sb = iv_sb[:, 0, 0:128]
            vals_sb = iv_sb[:, 1:1 + NKC, :]

            # table pre-loads: Exp and Sin live in different act-func sets;
            # touching both on a dummy at t~0 pulls both tables into the two
            # resident slots while the DMAs run.
            nc.gpsimd.memset(dumm[:], 0.0)
            nc.scalar.activation(dumo[:, 0:1], dumm[:], EXP)
            nc.scalar.activation(dumo[:, 1:2], dumm[:], SIN)

            # one transfer per engine DMA ring — rings serialize their own
            # transfers (~1KB/partition/us each), so spreading the inputs
            # across four rings lets all of them land in parallel.
            KA = H + 512
            nc.scalar.dma_start(out=qq_sb[:], in_=qq[:])
            nc.sync.dma_start(out=kk_sb[:, :, 0:KA], in_=kk[:, :, 0:KA])
            nc.gpsimd.dma_start(out=kk_sb[:, :, KA:], in_=kk[:, :, KA:])
            nc.gpsimd.dma_start(out=cn_sb[:], in_=cn[:])
            nc.gpsimd.dma_start(out=iv_sb[:], in_=iv[:])

            # sc[qb]: [128, 1024] f32 = 2 PSUM banks; matmuls write 512-col
            # halves (bank-aligned), exp reads the full 1024 in one call.
            sc = [ps_sc.tile([128, LK], F32, tag=f"sc{qb}", name=f"sc{qb}")
                  for qb in range(2)]

            with tc.tile_pool(name="ps_base", bufs=1, space="PSUM") as ps_base:
                # separate PSUM tiles per projection region: WAR tracking is
                # tile-granular, so a shared tile would serialize the k1
                # projections behind the DVE readers of k0/q.
                base_k0 = ps_base.tile([128, 512], F32, tag="bk0")
                base_k1 = ps_base.tile([128, 512], F32, tag="bk1")
                base_q = ps_base.tile([128, LQ], F32, tag="bq")

                def emit_fma(m, p48, lo, hi, src):
                    nc.vector.tensor_scalar(p48[:, 0, lo:hi], src,
                                            float(s_scale[m]), 48.0,
                                            MULT, ADD)
                    nc.vector.tensor_scalar(p48[:, 1, lo:hi], src,
                                            float(s_scale[m]), 48.0625,
                                            MULT, ADD)

                def emit_sin(m, p48, bas, lo, hi):
                    nc.scalar.activation(
                        bas[:, :, lo:hi],
                        p48[:].bitcast(U16)[:, :, 2 * lo:2 * hi:2],
                        SIN, scale=SCALE_SIN, bias=negpi_sb)

                def emit_qw(m, bas):
                    # qw[:,0] = cw*sin_q pairs with cos_k; qw[:,1] = cw*cos_q
                    qw = qwp.tile([128, 2, LQ], BF16, tag="qw",
                                  name=f"qw_{m}")
                    nc.vector.tensor_scalar_mul(qw[:], bas[:, :, LK:W],
                                                cw_sb[:, m:m + 1])
                    return qw

                def emit_scores(m, bas, qw):
                    for qb in range(2):
                        for half in range(2):
                            nc.tensor.matmul(
                                sc[qb][:, 512 * half:512 * (half + 1)],
                                qw[:, 0, 128 * qb:128 * (qb + 1)],
                                bas[:, 1, 512 * half:512 * (half + 1)],
                                start=(m == 0), stop=False)
                            nc.tensor.matmul(
                                sc[qb][:, 512 * half:512 * (half + 1)],
                                qw[:, 1, 128 * qb:128 * (qb + 1)],
                                bas[:, 0, 512 * half:512 * (half + 1)],
                                start=False, stop=(m == M - 1))

                def emit_scores0_half(half, bas, qw):
                    for qb in range(2):
                        nc.tensor.matmul(
                            sc[qb][:, 512 * half:512 * (half + 1)],
                            qw[:, 0, 128 * qb:128 * (qb + 1)],
                            bas[:, 1, 512 * half:512 * (half + 1)],
                            start=True, stop=False)
                        nc.tensor.matmul(
                            sc[qb][:, 512 * half:512 * (half + 1)],
                            qw[:, 1, 128 * qb:128 * (qb + 1)],
                            bas[:, 0, 512 * half:512 * (half + 1)],
                            start=False, stop=False)

                # base = [khT | qhT]. PE order follows DMA arrival: k half 0,
                # then q, then k half 1. The m=0 fma/sin chain is split per
                # region so the first score matmuls only wait on the half-0
                # sin while half 1 is still in the DMA.
                p48_0 = p48p.tile([128, 2, W], F32, tag="p48", name="p48_0")
                bas_0 = bp.tile([128, 2, W], BF16, tag="bas", name="bas_0")
                for d in range(2):
                    nc.tensor.matmul(base_q[:], wq_sb[:, d, :],
                                     qTd_sb[:, d, :],
                                     start=(d == 0), stop=(d == 1))
                for d in range(2):
                    nc.tensor.matmul(base_k0[:], wk_sb[:, d, :],
                                     kTd_sb[:, d, 0:512],
                                     start=(d == 0), stop=(d == 1))
                for d in range(2):
                    nc.tensor.matmul(base_k1[:], wk_sb[:, d, :],
                                     kTd_sb[:, d, 512:1024],
                                     start=(d == 0), stop=(d == 1))
                # m=0 q chain entirely on ACT (Identity-fma, sin, Copy-mul):
                # no cross-engine handoffs, and DVE is free to run the k
                # chain the moment the k projections land.
                IDENT = mybir.ActivationFunctionType.Identity
                nc.scalar.activation(p48_0[:, 0, LK:W], base_q[:], IDENT,
                                     scale=float(s_scale[0]), bias=b48_sb)
                nc.scalar.activation(p48_0[:, 1, LK:W], base_q[:], IDENT,
                                     scale=float(s_scale[0]), bias=b4806_sb)
                emit_sin(0, p48_0, bas_0, LK, W)
                qw_0 = qwp.tile([128, 2, LQ], BF16, tag="qw", name="qw_0")
                nc.scalar.activation(qw_0[:], bas_0[:, :, LK:W], COPY,
                                     scale=cw_sb[:, 0:1])
                emit_fma(0, p48_0, 0, 512, base_k0[:])
                emit_sin(0, p48_0, bas_0, 0, 512)
                emit_fma(0, p48_0, 512, 1024, base_k1[:])
                emit_sin(0, p48_0, bas_0, 512, 1024)
                emit_scores0_half(0, bas_0, qw_0)
                # PSUM->SBUF base copy in DVE slack: DVE reads PSUM at half
                # throughput, so the 2(M-1) steady-state fmas read SBUF.
                nc.vector.tensor_copy(base_sb[:, 0:512], base_k0[:])
                nc.vector.tensor_copy(base_sb[:, 512:1024], base_k1[:])
                nc.vector.tensor_copy(base_sb[:, LK:W], base_q[:])
                emit_scores0_half(1, bas_0, qw_0)

                def emit_qw8(m, bas8):
                    # DoubleRow pairs lhsT row i with rhs row i; the q rows
                    # are crossed so row0 = cw*cos_q meets sin_k and
                    # row1 = cw*sin_q meets cos_k.
                    qw8 = qwp.tile([128, 2, LQ], FP8, tag="qw8",
                                   name=f"qw8_{m}")
                    nc.vector.tensor_scalar_mul(qw8[:, 0, :],
                                                bas8[:, 1, LK:W],
                                                cw_sb[:, m:m + 1])
                    nc.vector.tensor_scalar_mul(qw8[:, 1, :],
                                                bas8[:, 0, LK:W],
                                                cw_sb[:, m:m + 1])
                    return qw8

                def emit_scores8(m, bas8, qw8):
                    for qb in range(2):
                        for half in range(2):
                            nc.tensor.matmul(
                                sc[qb][:, 512 * half:512 * (half + 1)],
                                qw8[:, :, 128 * qb:128 * (qb + 1)],
                                bas8[:, :, 512 * half:512 * (half + 1)],
                                start=False, stop=(m == M - 1),
                                perf_mode=mybir.MatmulPerfMode.DoubleRow)

                # software-pipelined: on the in-order DVE queue, the fma pair
                # of iteration m+1 is emitted before qw_m (which waits on the
                # ACT sin), so the fma feeding sin_{m+1} is never stuck
                # behind a cross-engine dependency. Terms m >= M_FP8 run the
                # score matmuls in fp8 DoubleRow (one matmul per sc half).
                prev, prev_qw = bas_0, qw_0
                prev_m = 0
                for m in range(1, M):
                    p48 = p48p.tile([128, 2, W], F32, tag="p48",
                                    name=f"p48_{m}")
                    emit_fma(m, p48, 0, W, base_sb[:])
                    if prev_m >= 1:
                        prev_qw = (emit_qw8(prev_m, prev)
                                   if prev_m >= M_FP8 else
                                   emit_qw(prev_m, prev))
                    if prev_m >= M_FP8:
                        emit_scores8(prev_m, prev, prev_qw)
                    else:
                        emit_scores(prev_m, prev, prev_qw)
                    bas = bp.tile([128, 2, W],
                                  FP8 if m >= M_FP8 else BF16,
                                  tag="bas8" if m >= M_FP8 else "bas",
                                  name=f"bas_{m}")
                    emit_sin(m, p48, bas, 0, W)
                    prev, prev_m = bas, m
                prev_qw = (emit_qw8(M - 1, prev) if M - 1 >= M_FP8
                           else emit_qw(M - 1, prev))
                if M - 1 >= M_FP8:
                    emit_scores8(M - 1, prev, prev_qw)
                else:
                    emit_scores(M - 1, prev, prev_qw)

            with tc.tile_pool(name="ps_tail", bufs=2, space="PSUM") as ps_tail:
                expS = [None, None]
                for qb in range(2):
                    expS[qb] = ep.tile([128, LK], BF16, tag="exps",
                                       name=f"expS{qb}")
                    nc.scalar.activation(expS[qb][:], sc[qb][:], EXP,
                                         scale=1.0 / 16.0)
                for qb in range(2):
                    # interleave transpose and AV accumulation: av_c runs as
                    # soon as chunk c's PSUM->SBUF copy lands, so the last
                    # AV matmul trails the last transpose by one chunk.
                    expT = etp.tile([128, NKC, 128], BF16, tag="expt")
                    av = ps_tail.tile([128, VS + 1], F32, tag="av",
                                      name=f"av{qb}")

                    def tpc(c, qb=qb, expT=expT):
                        tp = ps_tail.tile([128, 128], BF16, tag="tp",
                                          name=f"tp{qb}{c}")
                        nc.tensor.transpose(tp[:],
                                            expS[qb][:, 128 * c:128 * (c + 1)],
                                            ident_sb[:])
                        nc.vector.tensor_copy(expT[:, c, :], tp[:])

                    tpc(0)
                    tpc(1)
                    for c in range(NKC):
                        if c + 2 < NKC:
                            tpc(c + 2)
                        nc.tensor.matmul(av[:], expT[:, c, :], vals_sb[:, c, :],
                                         start=(c == 0), stop=(c == NKC - 1))
                    # numerator and denominator ship together; host divides.
                    o_sb = ep.tile([128, VS + 1], F32, tag="osb",
                                   name=f"osb{qb}")
                    if qb == 0:
                        nc.vector.tensor_copy(o_sb[:], av[:])
                        nc.sync.dma_start(out=out[0:128, :], in_=o_sb[:])
                    else:
                        nc.scalar.activation(o_sb[:], av[:], COPY)
                        nc.gpsimd.dma_start(out=out[128:256, :], in_=o_sb[:])

    nc.compile()
    return nc


def _pack_rows(a):
    # [256, N] -> [128, 2, N]: row r -> (r % 128, r // 128)
    return np.ascontiguousarray(a.reshape(2, 128, -1).transpose(1, 0, 2))


def _make_in_maps(inputs) -> list[dict]:
    queries = np.asarray(inputs["queries"], dtype=np.float32)
    key = np.asarray(inputs["key"], dtype=np.float32)
    value = np.asarray(inputs["value"], dtype=np.float32)
    vl = np.asarray(inputs["valid_length"], dtype=np.int32)
    W_q = np.asarray(inputs["W_q"], dtype=np.float32)
    W_k = np.asarray(inputs["W_k"], dtype=np.float32)
    W_v = np.asarray(inputs["W_v"], dtype=np.float32)

    cfit = np.asarray(C_FIT, np.float32)
    cn = np.empty((H, 3 + M), np.float32)
    cn[:, 0] = -math.pi
    cn[:, 1] = 48.0
    cn[:, 2] = 48.0625
    cn[:, 3:] = 16.0 * W_v[:, None] * cfit[None, :]
    cn = np.ascontiguousarray(cn)
    wk_b = _pack_rows(W_k.astype(ml_dtypes.bfloat16))
    wq_b = _pack_rows(W_q.astype(ml_dtypes.bfloat16))

    in_maps = []
    for b in range(B):
        v = max(int(vl[b]), 0)
        vals = np.zeros((LK, VS + 1), dtype=np.float32)
        vals[:v, :VS] = value[b, :v]
        vals[:v, VS] = 1.0
        iv = np.zeros((128, 1 + NKC, VS + 1), dtype=ml_dtypes.bfloat16)
        iv[:, 0, 0:128] = np.eye(128, dtype=ml_dtypes.bfloat16)
        iv[:, 1:, :] = vals.astype(ml_dtypes.bfloat16).reshape(
            NKC, 128, VS + 1).transpose(1, 0, 2)
        qq = np.concatenate(
            [wq_b, _pack_rows(queries[b].T.astype(ml_dtypes.bfloat16))],
            axis=2)
        kk = np.concatenate(
            [wk_b, _pack_rows(key[b].T.astype(ml_dtypes.bfloat16))],
            axis=2)
        in_maps.append({
            "qq": np.ascontiguousarray(qq),
            "kk": np.ascontiguousarray(kk),
            "cn": cn, "iv": np.ascontiguousarray(iv),
        })
    return in_maps


def _postprocess(res, inputs) -> np.ndarray:
    value = np.asarray(inputs["value"], dtype=np.float32)
    vl = np.asarray(inputs["valid_length"], dtype=np.int32)
    av = np.stack([np.asarray(res.results[i]["out"]) for i in range(B)], axis=0)
    with np.errstate(divide="ignore", invalid="ignore"):
        out = av[:, :, :VS] / av[:, :, VS:VS + 1]
    for b in range(B):
        if int(vl[b]) <= 0:
            out[b] = value[b].mean(axis=0, keepdims=True)
    return out.astype(np.float32)


def _sane(out, inputs) -> bool:
    # each output row is a convex combination of value rows, so it must be
    # finite and lie within the per-batch value range; a corrupted run
    # (transient device glitch) violates this with near-certainty.
    if not np.isfinite(out).all():
        return False
    value = np.asarray(inputs["value"], dtype=np.float32)
    bound = np.abs(value).max(axis=(1, 2)) * 1.05 + 0.1
    return bool((np.abs(out).max(axis=(1, 2)) <= bound).all())


def kernel(**inputs) -> np.ndarray:
    if "nc" not in _CACHE:
        _CACHE["nc"] = _build()
    nc = _CACHE["nc"]
    in_maps = _make_in_maps(inputs)
    for _attempt in range(3):
        res = run_bass_kernel_spmd(nc, in_maps, core_ids=list(range(B)))
        out = _postprocess(res, inputs)
        if _sane(out, inputs):
            break
    return out


# revision 47
# speedup vs baseline: 1.0718x; 1.0097x over previous
"""AdditiveAttention TRN2 kernel v8 — sin-basis scores via low-u16 binade
phase extraction, M=4 terms, fp8 DoubleRow for the small-coefficient terms.

scores[q,k] = sum_h W_v[h] tanh(qh+kh) with tanh(s) ~= sum_m c_m sin(w_m s)
factorized through sin(a+b) = sin a cos b + cos a sin b into 2M rank-128
matmuls per core (data-parallel over batch, one batch per NeuronCore).
Per-m pipeline:

  DVE  : p48_s = (w_m/8pi)*x + 48.0     f32 in binade [32,64): the low 16
  DVE  : p48_c = (w_m/8pi)*x + 48.0625  mantissa bits ARE the phase of
                                        w_m*x in 2^16 units (+1/16 value
                                        = +pi/2 phase for the cos row)
  ACT  : bas = Sin(lo16 * 2pi/2^16 - pi), reading the low u16 of each f32
         via a strided bitcast view — no range-reduction mask instructions
  DVE  : qw  = bas_q * (16*cw[m])       (per-partition scalar multiply)
  PE   : sc += qsw^T kc + qcw^T ks      m < 2: 8x 512-col bf16 matmuls
                                        m >= 2: 4x fp8e4 DoubleRow matmuls
                                        (both trig products in one pass)

The 16x weight scale keeps fp8 magnitudes healthy and is undone by the
Exp activation's scale=1/16. The base projections [khT | qhT] run once in
bf16 into three separate PSUM tiles (separate tiles because WAR tracking
is tile-granular), then copy to SBUF where DVE reads at 2 elem/cycle
(PSUM reads run at half rate). q (256) and k (1024) columns ride together
in [128, 1280]-wide ops. m=0 is split per region (q / k-half0 / k-half1)
with its q chain entirely on ACT, so the first score matmuls launch while
the second half of the k DMA is still in flight. Emission is
software-pipelined: the fma pair of iteration m+1 is emitted before qw_m
on the in-order DVE queue. Inputs arrive as three packed DMAs spread
across the SP/ACT/Pool rings; dummy Exp/Sin activations at t~0 pre-load
both activation tables. The softmax tail transposes exp(sc) via the PE,
interleaving transposes with the attn@V accumulation; numerator and
denominator ship to HBM together and the host divides.
"""

import math

import ml_dtypes
import numpy as np

from concourse import bacc, mybir
from concourse import tile
from concourse.bass_utils import run_bass_kernel_spmd

B, LQ, LK, QS, KS, H, VS = 8, 256, 1024, 256, 256, 128, 256
F32 = mybir.dt.float32
BF16 = mybir.dt.bfloat16
FP8 = mybir.dt.float8e4
M_FP8 = 2  # terms >= this index use fp8 DoubleRow score matmuls

W_FIT = [0.29237, 0.87651, 1.51083, 2.50362]
C_FIT = [1.23737, 0.30825, 0.14462, 0.04779]
M = len(W_FIT)

SCALE_SIN = 2.0 * math.pi / (1 << 16)
NKC = LK // 128         # 8 key chunks of 128
W = LK + LQ             # 1280: k columns then q columns

_CACHE: dict = {}


def _build():
    nc = bacc.Bacc("TRN2", target_bir_lowering=False, debug=False)
    # packed params: fewer DMAs (each DMA costs ~620ns of queue issue time)
    qq = nc.declare_dram_parameter("qq", [128, 2, H + LQ], BF16, isOutput=False)
    kk = nc.declare_dram_parameter("kk", [128, 2, H + LK], BF16, isOutput=False)
    cn = nc.declare_dram_parameter("cn", [H, 3 + M], F32, isOutput=False)
    iv = nc.declare_dram_parameter("iv", [128, 1 + NKC, VS + 1], BF16,
                                   isOutput=False)
    # av rows: [sum_k attn*v | sum_k attn]; the division happens on host
    out = nc.declare_dram_parameter("out", [LQ, VS + 1], F32, isOutput=True)

    SIN = mybir.ActivationFunctionType.Sin
    EXP = mybir.ActivationFunctionType.Exp
    COPY = mybir.ActivationFunctionType.Copy
    ADD = mybir.AluOpType.add
    MULT = mybir.AluOpType.mult
    U16 = mybir.dt.uint16

    s_scale = [w / (8.0 * math.pi) for w in W_FIT]

    with tile.TileContext(nc) as tc:
        with (
            tc.tile_pool(name="const", bufs=1) as cpool,
            tc.tile_pool(name="p48p", bufs=3) as p48p,
            tc.tile_pool(name="bp", bufs=3) as bp,
            tc.tile_pool(name="qwp", bufs=3) as qwp,
            tc.tile_pool(name="ep", bufs=2) as ep,
            tc.tile_pool(name="etp", bufs=2) as etp,
            tc.tile_pool(name="ps_sc", bufs=1, space="PSUM") as ps_sc,
        ):
            qq_sb = cpool.tile([128, 2, H + LQ], BF16)
            kk_sb = cpool.tile([128, 2, H + LK], BF16)
            cn_sb = cpool.tile([128, 3 + M], F32)
            iv_sb = cpool.tile([128, 1 + NKC, VS + 1], BF16)
            base_sb = cpool.tile([128, W], F32)
            dumm = cpool.tile([128, 1], F32)
            dumo = cpool.tile([128, 2], F32)
            wq_sb = qq_sb[:, :, 0:H]
            qTd_sb = qq_sb[:, :, H:H + LQ]
            wk_sb = kk_sb[:, :, 0:H]
            kTd_sb = kk_sb[:, :, H:H + LK]
            negpi_sb = cn_sb[:, 0:1]
            b48_sb = cn_sb[:, 1:2]
            b4806_sb = cn_sb[:, 2:3]
            cw_sb = cn_sb[:, 3:3 + M]
            ident_sb = iv_sb[:, 0, 0:128]
            vals_sb = iv_sb[:, 1:1 + NKC, :]

            # table pre-loads: Exp and Sin live in different act-func sets;
            # touching both on a dummy at t~0 pulls both tables into the two
            # resident slots while the DMAs run.
            nc.gpsimd.memset(dumm[:], 0.0)
            nc.scalar.activation(dumo[:, 0:1], dumm[:], EXP)
            nc.scalar.activation(dumo[:, 1:2], dumm[:], SIN)

            # one transfer per engine DMA ring — rings serialize their own
            # transfers (~1KB/partition/us each), so spreading the inputs
            # across four rings lets all of them land in parallel.
            KA = H + 512
            nc.scalar.dma_start(out=qq_sb[:], in_=qq[:])
            nc.sync.dma_start(out=kk_sb[:, :, 0:KA], in_=kk[:, :, 0:KA])
            nc.gpsimd.dma_start(out=kk_sb[:, :, KA:], in_=kk[:, :, KA:])
            nc.gpsimd.dma_start(out=cn_sb[:], in_=cn[:])
            nc.gpsimd.dma_start(out=iv_sb[:], in_=iv[:])

            # sc[qb]: [128, 1024] f32 = 2 PSUM banks; matmuls write 512-col
            # halves (bank-aligned), exp reads the full 1024 in one call.
            sc = [ps_sc.tile([128, LK], F32, tag=f"sc{qb}", name=f"sc{qb}")
                  for qb in range(2)]

            with tc.tile_pool(name="ps_base", bufs=1, space="PSUM") as ps_base:
                # separate PSUM tiles per projection region: WAR tracking is
                # tile-granular, so a shared tile would serialize the k1
                # projections behind the DVE readers of k0/q.
                base_k0 = ps_base.tile([128, 512], F32, tag="bk0")
                base_k1 = ps_base.tile([128, 512], F32, tag="bk1")
                base_q = ps_base.tile([128, LQ], F32, tag="bq")

                def emit_fma(m, p48, lo, hi, src):
                    nc.vector.tensor_scalar(p48[:, 0, lo:hi], src,
                                            float(s_scale[m]), 48.0,
                                            MULT, ADD)
                    nc.vector.tensor_scalar(p48[:, 1, lo:hi], src,
                                            float(s_scale[m]), 48.0625,
                                            MULT, ADD)

                def emit_sin(m, p48, bas, lo, hi):
                    nc.scalar.activation(
                        bas[:, :, lo:hi],
                        p48[:].bitcast(U16)[:, :, 2 * lo:2 * hi:2],
                        SIN, scale=SCALE_SIN, bias=negpi_sb)

                def emit_qw(m, bas):
                    # qw[:,0] = cw*sin_q pairs with cos_k; qw[:,1] = cw*cos_q
                    qw = qwp.tile([128, 2, LQ], BF16, tag="qw",
                                  name=f"qw_{m}")
                    nc.vector.tensor_scalar_mul(qw[:], bas[:, :, LK:W],
                                                cw_sb[:, m:m + 1])
                    return qw

                def emit_scores(m, bas, qw):
                    for qb in range(2):
                        for half in range(2):
                            nc.tensor.matmul(
                                sc[qb][:, 512 * half:512 * (half + 1)],
                                qw[:, 0, 128 * qb:128 * (qb + 1)],
                                bas[:, 1, 512 * half:512 * (half + 1)],
                                start=(m == 0), stop=False)
                            nc.tensor.matmul(
                                sc[qb][:, 512 * half:512 * (half + 1)],
                                qw[:, 1, 128 * qb:128 * (qb + 1)],
                                bas[:, 0, 512 * half:512 * (half + 1)],
                                start=False, stop=(m == M - 1))

                def emit_scores0_half(half, bas, qw):
                    for qb in range(2):
                        nc.tensor.matmul(
                            sc[qb][:, 512 * half:512 * (half + 1)],
                            qw[:, 0, 128 * qb:128 * (qb + 1)],
                            bas[:, 1, 512 * half:512 * (half + 1)],
                            start=True, stop=False)
                        nc.tensor.matmul(
                            sc[qb][:, 512 * half:512 * (half + 1)],
                            qw[:, 1, 128 * qb:128 * (qb + 1)],
                            bas[:, 0, 512 * half:512 * (half + 1)],
                            start=False, stop=False)

                # base = [khT | qhT]. PE order follows DMA arrival: k half 0,
                # then q, then k half 1. The m=0 fma/sin chain is split per
                # region so the first score matmuls only wait on the half-0
                # sin while half 1 is still in the DMA.
                p48_0 = p48p.tile([128, 2, W], F32, tag="p48", name="p48_0")
                bas_0 = bp.tile([128, 2, W], BF16, tag="bas", name="bas_0")
                for d in range(2):
                    nc.tensor.matmul(base_q[:], wq_sb[:, d, :],
                                     qTd_sb[:, d, :],
                                     start=(d == 0), stop=(d == 1))
                for d in range(2):
                    nc.tensor.matmul(base_k0[:], wk_sb[:, d, :],
                                     kTd_sb[:, d, 0:512],
                                     start=(d == 0), stop=(d == 1))
                for d in range(2):
                    nc.tensor.matmul(base_k1[:], wk_sb[:, d, :],
                                     kTd_sb[:, d, 512:1024],
                                     start=(d == 0), stop=(d == 1))
                # m=0 q chain entirely on ACT (Identity-fma, sin, Copy-mul):
                # no cross-engine handoffs, and DVE is free to run the k
                # chain the moment the k projections land.
                IDENT = mybir.ActivationFunctionType.Identity
                nc.scalar.activation(p48_0[:, 0, LK:W], base_q[:], IDENT,
                                     scale=float(s_scale[0]), bias=b48_sb)
                nc.scalar.activation(p48_0[:, 1, LK:W], base_q[:], IDENT,
                                     scale=float(s_scale[0]), bias=b4806_sb)
                emit_sin(0, p48_0, bas_0, LK, W)
                qw_0 = qwp.tile([128, 2, LQ], BF16, tag="qw", name="qw_0")
                nc.scalar.activation(qw_0[:], bas_0[:, :, LK:W], COPY,
                                     scale=cw_sb[:, 0:1])
                # PSUM->SBUF base copies feed the steady-state fmas (DVE
                # reads PSUM at half throughput): q lands first, and each k
                # copy follows its region's m=0 fma so fma_1 starts ASAP.
                nc.vector.tensor_copy(base_sb[:, LK:W], base_q[:])
                emit_fma(0, p48_0, 0, 512, base_k0[:])
                emit_sin(0, p48_0, bas_0, 0, 512)
                nc.vector.tensor_copy(base_sb[:, 0:512], base_k0[:])
                emit_fma(0, p48_0, 512, 1024, base_k1[:])
                emit_sin(0, p48_0, bas_0, 512, 1024)
                emit_scores0_half(0, bas_0, qw_0)
                nc.vector.tensor_copy(base_sb[:, 512:1024], base_k1[:])
                emit_scores0_half(1, bas_0, qw_0)

                def emit_qw8(m, bas8):
                    # DoubleRow pairs lhsT row i with rhs row i; the q rows
                    # are crossed so row0 = cw*cos_q meets sin_k and
                    # row1 = cw*sin_q meets cos_k.
                    qw8 = qwp.tile([128, 2, LQ], FP8, tag="qw8",
                                   name=f"qw8_{m}")
                    nc.vector.tensor_scalar_mul(qw8[:, 0, :],
                                                bas8[:, 1, LK:W],
                                                cw_sb[:, m:m + 1])
                    nc.vector.tensor_scalar_mul(qw8[:, 1, :],
                                                bas8[:, 0, LK:W],
                                                cw_sb[:, m:m + 1])
                    return qw8

                def emit_scores8(m, bas8, qw8):
                    for qb in range(2):
                        for half in range(2):
                            nc.tensor.matmul(
                                sc[qb][:, 512 * half:512 * (half + 1)],
                                qw8[:, :, 128 * qb:128 * (qb + 1)],
                                bas8[:, :, 512 * half:512 * (half + 1)],
                                start=False, stop=(m == M - 1),
                                perf_mode=mybir.MatmulPerfMode.DoubleRow)

                # software-pipelined: on the in-order DVE queue, the fma pair
                # of iteration m+1 is emitted before qw_m (which waits on the
                # ACT sin), so the fma feeding sin_{m+1} is never stuck
                # behind a cross-engine dependency. Terms m >= M_FP8 run the
                # score matmuls in fp8 DoubleRow (one matmul per sc half).
                prev, prev_qw = bas_0, qw_0
                prev_m = 0
                for m in range(1, M):
                    p48 = p48p.tile([128, 2, W], F32, tag="p48",
                                    name=f"p48_{m}")
                    emit_fma(m, p48, 0, W, base_sb[:])
                    if prev_m >= 1:
                        prev_qw = (emit_qw8(prev_m, prev)
                                   if prev_m >= M_FP8 else
                                   emit_qw(prev_m, prev))
                    if prev_m >= M_FP8:
                        emit_scores8(prev_m, prev, prev_qw)
                    else:
                        emit_scores(prev_m, prev, prev_qw)
                    bas = bp.tile([128, 2, W],
                                  FP8 if m >= M_FP8 else BF16,
                                  tag="bas8" if m >= M_FP8 else "bas",
                                  name=f"bas_{m}")
                    emit_sin(m, p48, bas, 0, W)
                    prev, prev_m = bas, m
                prev_qw = (emit_qw8(M - 1, prev) if M - 1 >= M_FP8
                           else emit_qw(M - 1, prev))
                if M - 1 >= M_FP8:
                    emit_scores8(M - 1, prev, prev_qw)
                else:
                    emit_scores(M - 1, prev, prev_qw)

            with tc.tile_pool(name="ps_tail", bufs=2, space="PSUM") as ps_tail:
                expS = [None, None]
                for qb in range(2):
                    expS[qb] = ep.tile([128, LK], BF16, tag="exps",
                                       name=f"expS{qb}")
                    for half in range(2):
                        nc.scalar.activation(
                            expS[qb][:, 512 * half:512 * (half + 1)],
                            sc[qb][:, 512 * half:512 * (half + 1)], EXP,
                            scale=1.0 / 16.0)
                for qb in range(2):
                    # interleave transpose and AV accumulation: av_c runs as
                    # soon as chunk c's PSUM->SBUF copy lands, so the last
                    # AV matmul trails the last transpose by one chunk.
                    expT = etp.tile([128, NKC, 128], BF16, tag="expt")
                    av = ps_tail.tile([128, VS + 1], F32, tag="av",
                                      name=f"av{qb}")

                    def tpc(c, qb=qb, expT=expT):
                        tp = ps_tail.tile([128, 128], BF16, tag="tp",
                                          name=f"tp{qb}{c}")
                        nc.tensor.transpose(tp[:],
                                            expS[qb][:, 128 * c:128 * (c + 1)],
                                            ident_sb[:])
                        nc.vector.tensor_copy(expT[:, c, :], tp[:])

                    tpc(0)
                    tpc(1)
                    for c in range(NKC):
                        if c + 2 < NKC:
                            tpc(c + 2)
                        nc.tensor.matmul(av[:], expT[:, c, :], vals_sb[:, c, :],
                                         start=(c == 0), stop=(c == NKC - 1))
                    # numerator and denominator ship together; host divides.
                    o_sb = ep.tile([128, VS + 1], F32, tag="osb",
                                   name=f"osb{qb}")
                    if qb == 0:
                        nc.vector.tensor_copy(o_sb[:], av[:])
                        nc.sync.dma_start(out=out[0:128, :], in_=o_sb[:])
                    else:
                        nc.scalar.activation(o_sb[:], av[:], COPY)
                        nc.gpsimd.dma_start(out=out[128:256, :], in_=o_sb[:])

    nc.compile()
    return nc


def _pack_rows(a):
    # [256, N] -> [128, 2, N]: row r -> (r % 128, r // 128)
    return np.ascontiguousarray(a.reshape(2, 128, -1).transpose(1, 0, 2))


def _make_in_maps(inputs) -> list[dict]:
    queries = np.asarray(inputs["queries"], dtype=np.float32)
    key = np.asarray(inputs["key"], dtype=np.float32)
    value = np.asarray(inputs["value"], dtype=np.float32)
    vl = np.asarray(inputs["valid_length"], dtype=np.int32)
    W_q = np.asarray(inputs["W_q"], dtype=np.float32)
    W_k = np.asarray(inputs["W_k"], dtype=np.float32)
    W_v = np.asarray(inputs["W_v"], dtype=np.float32)

    cfit = np.asarray(C_FIT, np.float32)
    cn = np.empty((H, 3 + M), np.float32)
    cn[:, 0] = -math.pi
    cn[:, 1] = 48.0
    cn[:, 2] = 48.0625
    cn[:, 3:] = 16.0 * W_v[:, None] * cfit[None, :]
    cn = np.ascontiguousarray(cn)
    wk_b = _pack_rows(W_k.astype(ml_dtypes.bfloat16))
    wq_b = _pack_rows(W_q.astype(ml_dtypes.bfloat16))

    in_maps = []
    for b in range(B):
        v = max(int(vl[b]), 0)
        vals = np.zeros((LK, VS + 1), dtype=np.float32)
        vals[:v, :VS] = value[b, :v]
        vals[:v, VS] = 1.0
        iv = np.zeros((128, 1 + NKC, VS + 1), dtype=ml_dtypes.bfloat16)
        iv[:, 0, 0:128] = np.eye(128, dtype=ml_dtypes.bfloat16)
        iv[:, 1:, :] = vals.astype(ml_dtypes.bfloat16).reshape(
            NKC, 128, VS + 1).transpose(1, 0, 2)
        qq = np.concatenate(
            [wq_b, _pack_rows(queries[b].T.astype(ml_dtypes.bfloat16))],
            axis=2)
        kk = np.concatenate(
            [wk_b, _pack_rows(key[b].T.astype(ml_dtypes.bfloat16))],
            axis=2)
        in_maps.append({
            "qq": np.ascontiguousarray(qq),
            "kk": np.ascontiguousarray(kk),
            "cn": cn, "iv": np.ascontiguousarray(iv),
        })
    return in_maps


def _postprocess(res, inputs) -> np.ndarray:
    value = np.asarray(inputs["value"], dtype=np.float32)
    vl = np.asarray(inputs["valid_length"], dtype=np.int32)
    av = np.stack([np.asarray(res.results[i]["out"]) for i in range(B)], axis=0)
    with np.errstate(divide="ignore", invalid="ignore"):
        out = av[:, :, :VS] / av[:, :, VS:VS + 1]
    for b in range(B):
        if int(vl[b]) <= 0:
            out[b] = value[b].mean(axis=0, keepdims=True)
    return out.astype(np.float32)


def _sane(out, inputs) -> bool:
    # each output row is a convex combination of value rows, so it must be
    # finite and lie within the per-batch value range; a corrupted run
    # (transient device glitch) violates this with near-certainty.
    if not np.isfinite(out).all():
        return False
    value = np.asarray(inputs["value"], dtype=np.float32)
    bound = np.abs(value).max(axis=(1, 2)) * 1.05 + 0.1
    return bool((np.abs(out).max(axis=(1, 2)) <= bound).all())


def kernel(**inputs) -> np.ndarray:
    if "nc" not in _CACHE:
        _CACHE["nc"] = _build()
    nc = _CACHE["nc"]
    in_maps = _make_in_maps(inputs)
    for _attempt in range(3):
        res = run_bass_kernel_spmd(nc, in_maps, core_ids=list(range(B)))
        out = _postprocess(res, inputs)
        if _sane(out, inputs):
            break
    return out


# revision 48
# speedup vs baseline: 1.0846x; 1.0119x over previous
"""AdditiveAttention TRN2 kernel v8 — sin-basis scores via low-u16 binade
phase extraction, M=4 terms, fp8 DoubleRow for the small-coefficient terms.

scores[q,k] = sum_h W_v[h] tanh(qh+kh) with tanh(s) ~= sum_m c_m sin(w_m s)
factorized through sin(a+b) = sin a cos b + cos a sin b into 2M rank-128
matmuls per core (data-parallel over batch, one batch per NeuronCore).
Per-m pipeline:

  DVE  : p48_s = (w_m/8pi)*x + 48.0     f32 in binade [32,64): the low 16
  DVE  : p48_c = (w_m/8pi)*x + 48.0625  mantissa bits ARE the phase of
                                        w_m*x in 2^16 units (+1/16 value
                                        = +pi/2 phase for the cos row)
  ACT  : bas = Sin(lo16 * 2pi/2^16 - pi), reading the low u16 of each f32
         via a strided bitcast view — no range-reduction mask instructions
  DVE  : qw  = bas_q * (16*cw[m])       (per-partition scalar multiply)
  PE   : sc += qsw^T kc + qcw^T ks      m < 2: 8x 512-col bf16 matmuls
                                        m >= 2: 4x fp8e4 DoubleRow matmuls
                                        (both trig products in one pass)

The 16x weight scale keeps fp8 magnitudes healthy and is undone by the
Exp activation's scale=1/16. The base projections [khT | qhT] run once in
bf16 into three separate PSUM tiles (separate tiles because WAR tracking
is tile-granular), then copy to SBUF where DVE reads at 2 elem/cycle
(PSUM reads run at half rate). q (256) and k (1024) columns ride together
in [128, 1280]-wide ops. m=0 is split per region (q / k-half0 / k-half1)
with its q chain entirely on ACT, so the first score matmuls launch while
the second half of the k DMA is still in flight. Emission is
software-pipelined: the fma pair of iteration m+1 is emitted before qw_m
on the in-order DVE queue. Inputs arrive as three packed DMAs spread
across the SP/ACT/Pool rings; dummy Exp/Sin activations at t~0 pre-load
both activation tables. The softmax tail transposes exp(sc) via the PE,
interleaving transposes with the attn@V accumulation; numerator and
denominator ship to HBM together and the host divides.
"""

import math

import ml_dtypes
import numpy as np

from concourse import bacc, mybir
from concourse import tile
from concourse.bass_utils import run_bass_kernel_spmd

B, LQ, LK, QS, KS, H, VS = 8, 256, 1024, 256, 256, 128, 256
F32 = mybir.dt.float32
BF16 = mybir.dt.bfloat16
FP8 = mybir.dt.float8e4
M_FP8 = 2  # terms >= this index use fp8 DoubleRow score matmuls

W_FIT = [0.29237, 0.87651, 1.51083, 2.50362]
C_FIT = [1.23737, 0.30825, 0.14462, 0.04779]
M = len(W_FIT)

SCALE_SIN = 2.0 * math.pi / (1 << 16)
NKC = LK // 128         # 8 key chunks of 128
W = LK + LQ             # 1280: k columns then q columns

_CACHE: dict = {}


def _build():
    nc = bacc.Bacc("TRN2", target_bir_lowering=False, debug=False)
    # packed params: fewer DMAs (each DMA costs ~620ns of queue issue time)
    qq = nc.declare_dram_parameter("qq", [128, 2, H + LQ], BF16, isOutput=False)
    kk = nc.declare_dram_parameter("kk", [128, 2, H + LK], BF16, isOutput=False)
    cn = nc.declare_dram_parameter("cn", [H, 3 + M], F32, isOutput=False)
    iv = nc.declare_dram_parameter("iv", [128, 1 + NKC, VS + 1], BF16,
                                   isOutput=False)
    # av rows: [sum_k attn*v | sum_k attn]; the division happens on host
    out = nc.declare_dram_parameter("out", [LQ, VS + 1], F32, isOutput=True)

    SIN = mybir.ActivationFunctionType.Sin
    EXP = mybir.ActivationFunctionType.Exp
    COPY = mybir.ActivationFunctionType.Copy
    ADD = mybir.AluOpType.add
    MULT = mybir.AluOpType.mult
    U16 = mybir.dt.uint16

    s_scale = [w / (8.0 * math.pi) for w in W_FIT]

    with tile.TileContext(nc) as tc:
        with (
            tc.tile_pool(name="const", bufs=1) as cpool,
            tc.tile_pool(name="p48p", bufs=3) as p48p,
            tc.tile_pool(name="bp", bufs=3) as bp,
            tc.tile_pool(name="qwp", bufs=3) as qwp,
            tc.tile_pool(name="ep", bufs=2) as ep,
            tc.tile_pool(name="etp", bufs=2) as etp,
            tc.tile_pool(name="ps_sc", bufs=1, space="PSUM") as ps_sc,
        ):
            qq_sb = cpool.tile([128, 2, H + LQ], BF16)
            kk_sb = cpool.tile([128, 2, H + LK], BF16)
            cn_sb = cpool.tile([128, 3 + M], F32)
            iv_sb = cpool.tile([128, 1 + NKC, VS + 1], BF16)
            base_sb = cpool.tile([128, W], F32)
            dumm = cpool.tile([128, 1], F32)
            dumo = cpool.tile([128, 2], F32)
            wq_sb = qq_sb[:, :, 0:H]
            qTd_sb = qq_sb[:, :, H:H + LQ]
            wk_sb = kk_sb[:, :, 0:H]
            kTd_sb = kk_sb[:, :, H:H + LK]
            negpi_sb = cn_sb[:, 0:1]
            b48_sb = cn_sb[:, 1:2]
            b4806_sb = cn_sb[:, 2:3]
            cw_sb = cn_sb[:, 3:3 + M]
            ident_sb = iv_sb[:, 0, 0:128]
            vals_sb = iv_sb[:, 1:1 + NKC, :]

            # table pre-loads: Exp and Sin live in different act-func sets;
            # touching both on a dummy at t~0 pulls both tables into the two
            # resident slots while the DMAs run.
            nc.gpsimd.memset(dumm[:], 0.0)
            nc.scalar.activation(dumo[:, 0:1], dumm[:], EXP)
            nc.scalar.activation(dumo[:, 1:2], dumm[:], SIN)

            # one transfer per engine DMA ring — rings serialize their own
            # transfers (~1KB/partition/us each), so spreading the inputs
            # across four rings lets all of them land in parallel.
            KA = H + 512
            nc.scalar.dma_start(out=qq_sb[:], in_=qq[:])
            nc.sync.dma_start(out=kk_sb[:, :, 0:KA], in_=kk[:, :, 0:KA])
            nc.gpsimd.dma_start(out=kk_sb[:, :, KA:], in_=kk[:, :, KA:])
            nc.gpsimd.dma_start(out=cn_sb[:], in_=cn[:])
            nc.gpsimd.dma_start(out=iv_sb[:], in_=iv[:])

            # sc[qb]: [128, 1024] f32 = 2 PSUM banks; matmuls write 512-col
            # halves (bank-aligned), exp reads the full 1024 in one call.
            sc = [ps_sc.tile([128, LK], F32, tag=f"sc{qb}", name=f"sc{qb}")
                  for qb in range(2)]

            with tc.tile_pool(name="ps_base", bufs=1, space="PSUM") as ps_base:
                # separate PSUM tiles per projection region: WAR tracking is
                # tile-granular, so a shared tile would serialize the k1
                # projections behind the DVE readers of k0/q.
                base_k0 = ps_base.tile([128, 512], F32, tag="bk0")
                base_k1 = ps_base.tile([128, 512], F32, tag="bk1")
                base_q = ps_base.tile([128, LQ], F32, tag="bq")

                def emit_fma(m, p48, lo, hi, src):
                    nc.vector.tensor_scalar(p48[:, 0, lo:hi], src,
                                            float(s_scale[m]), 48.0,
                                            MULT, ADD)
                    nc.vector.tensor_scalar(p48[:, 1, lo:hi], src,
                                            float(s_scale[m]), 48.0625,
                                            MULT, ADD)

                def emit_sin(m, p48, bas, lo, hi):
                    nc.scalar.activation(
                        bas[:, :, lo:hi],
                        p48[:].bitcast(U16)[:, :, 2 * lo:2 * hi:2],
                        SIN, scale=SCALE_SIN, bias=negpi_sb)

                def emit_qw(m, bas):
                    # qw[:,0] = cw*sin_q pairs with cos_k; qw[:,1] = cw*cos_q
                    qw = qwp.tile([128, 2, LQ], BF16, tag="qw",
                                  name=f"qw_{m}")
                    nc.vector.tensor_scalar_mul(qw[:], bas[:, :, LK:W],
                                                cw_sb[:, m:m + 1])
                    return qw

                def emit_scores(m, bas, qw):
                    for qb in range(2):
                        for half in range(2):
                            nc.tensor.matmul(
                                sc[qb][:, 512 * half:512 * (half + 1)],
                                qw[:, 0, 128 * qb:128 * (qb + 1)],
                                bas[:, 1, 512 * half:512 * (half + 1)],
                                start=(m == 0), stop=False)
                            nc.tensor.matmul(
                                sc[qb][:, 512 * half:512 * (half + 1)],
                                qw[:, 1, 128 * qb:128 * (qb + 1)],
                                bas[:, 0, 512 * half:512 * (half + 1)],
                                start=False, stop=(m == M - 1))

                def emit_scores0_half(half, bas, qw):
                    for qb in range(2):
                        nc.tensor.matmul(
                            sc[qb][:, 512 * half:512 * (half + 1)],
                            qw[:, 0, 128 * qb:128 * (qb + 1)],
                            bas[:, 1, 512 * half:512 * (half + 1)],
                            start=True, stop=False)
                        nc.tensor.matmul(
                            sc[qb][:, 512 * half:512 * (half + 1)],
                            qw[:, 1, 128 * qb:128 * (qb + 1)],
                            bas[:, 0, 512 * half:512 * (half + 1)],
                            start=False, stop=False)

                # base = [khT | qhT]. PE order follows DMA arrival: k half 0,
                # then q, then k half 1. The m=0 fma/sin chain is split per
                # region so the first score matmuls only wait on the half-0
                # sin while half 1 is still in the DMA.
                p48_0 = p48p.tile([128, 2, W], F32, tag="p48", name="p48_0")
                bas_0 = bp.tile([128, 2, W], BF16, tag="bas", name="bas_0")
                for d in range(2):
                    nc.tensor.matmul(base_q[:], wq_sb[:, d, :],
                                     qTd_sb[:, d, :],
                                     start=(d == 0), stop=(d == 1))
                for d in range(2):
                    nc.tensor.matmul(base_k0[:], wk_sb[:, d, :],
                                     kTd_sb[:, d, 0:512],
                                     start=(d == 0), stop=(d == 1))
                for d in range(2):
                    nc.tensor.matmul(base_k1[:], wk_sb[:, d, :],
                                     kTd_sb[:, d, 512:1024],
                                     start=(d == 0), stop=(d == 1))
                # m=0 q chain entirely on ACT (Identity-fma, sin, Copy-mul):
                # no cross-engine handoffs, and DVE is free to run the k
                # chain the moment the k projections land.
                IDENT = mybir.ActivationFunctionType.Identity
                nc.scalar.activation(p48_0[:, 0, LK:W], base_q[:], IDENT,
                                     scale=float(s_scale[0]), bias=b48_sb)
                nc.scalar.activation(p48_0[:, 1, LK:W], base_q[:], IDENT,
                                     scale=float(s_scale[0]), bias=b4806_sb)
                emit_sin(0, p48_0, bas_0, LK, W)
                qw_0 = qwp.tile([128, 2, LQ], BF16, tag="qw", name="qw_0")
                nc.scalar.activation(qw_0[:], bas_0[:, :, LK:W], COPY,
                                     scale=cw_sb[:, 0:1])
                # copy-first, then fma from SBUF: one PSUM-rate pass per
                # region (the copy) instead of two, so the DVE serial chain
                # feeding sin_1 is ~1us shorter.
                nc.vector.tensor_copy(base_sb[:, LK:W], base_q[:])
                nc.vector.tensor_copy(base_sb[:, 0:512], base_k0[:])
                emit_fma(0, p48_0, 0, 512, base_sb[:, 0:512])
                emit_sin(0, p48_0, bas_0, 0, 512)
                nc.vector.tensor_copy(base_sb[:, 512:1024], base_k1[:])
                emit_fma(0, p48_0, 512, 1024, base_sb[:, 512:1024])
                emit_sin(0, p48_0, bas_0, 512, 1024)
                emit_scores0_half(0, bas_0, qw_0)
                emit_scores0_half(1, bas_0, qw_0)

                def emit_qw8(m, bas8):
                    # DoubleRow pairs lhsT row i with rhs row i; the q rows
                    # are crossed so row0 = cw*cos_q meets sin_k and
                    # row1 = cw*sin_q meets cos_k.
                    qw8 = qwp.tile([128, 2, LQ], FP8, tag="qw8",
                                   name=f"qw8_{m}")
                    nc.vector.tensor_scalar_mul(qw8[:, 0, :],
                                                bas8[:, 1, LK:W],
                                                cw_sb[:, m:m + 1])
                    nc.vector.tensor_scalar_mul(qw8[:, 1, :],
                                                bas8[:, 0, LK:W],
                                                cw_sb[:, m:m + 1])
                    return qw8

                def emit_scores8(m, bas8, qw8):
                    for qb in range(2):
                        for half in range(2):
                            nc.tensor.matmul(
                                sc[qb][:, 512 * half:512 * (half + 1)],
                                qw8[:, :, 128 * qb:128 * (qb + 1)],
                                bas8[:, :, 512 * half:512 * (half + 1)],
                                start=False, stop=(m == M - 1),
                                perf_mode=mybir.MatmulPerfMode.DoubleRow)

                # software-pipelined: on the in-order DVE queue, the fma pair
                # of iteration m+1 is emitted before qw_m (which waits on the
                # ACT sin), so the fma feeding sin_{m+1} is never stuck
                # behind a cross-engine dependency. Terms m >= M_FP8 run the
                # score matmuls in fp8 DoubleRow (one matmul per sc half).
                prev, prev_qw = bas_0, qw_0
                prev_m = 0
                for m in range(1, M):
                    p48 = p48p.tile([128, 2, W], F32, tag="p48",
                                    name=f"p48_{m}")
                    emit_fma(m, p48, 0, W, base_sb[:])
                    if prev_m >= 1:
                        prev_qw = (emit_qw8(prev_m, prev)
                                   if prev_m >= M_FP8 else
                                   emit_qw(prev_m, prev))
                    if prev_m >= M_FP8:
                        emit_scores8(prev_m, prev, prev_qw)
                    else:
                        emit_scores(prev_m, prev, prev_qw)
                    bas = bp.tile([128, 2, W],
                                  FP8 if m >= M_FP8 else BF16,
                                  tag="bas8" if m >= M_FP8 else "bas",
                                  name=f"bas_{m}")
                    emit_sin(m, p48, bas, 0, W)
                    prev, prev_m = bas, m
                prev_qw = (emit_qw8(M - 1, prev) if M - 1 >= M_FP8
                           else emit_qw(M - 1, prev))
                if M - 1 >= M_FP8:
                    emit_scores8(M - 1, prev, prev_qw)
                else:
                    emit_scores(M - 1, prev, prev_qw)

            with tc.tile_pool(name="ps_tail", bufs=2, space="PSUM") as ps_tail:
                expS = [None, None]
                for qb in range(2):
                    expS[qb] = ep.tile([128, LK], BF16, tag="exps",
                                       name=f"expS{qb}")
                    for half in range(2):
                        nc.scalar.activation(
                            expS[qb][:, 512 * half:512 * (half + 1)],
                            sc[qb][:, 512 * half:512 * (half + 1)], EXP,
                            scale=1.0 / 16.0)
                for qb in range(2):
                    # interleave transpose and AV accumulation: av_c runs as
                    # soon as chunk c's PSUM->SBUF copy lands, so the last
                    # AV matmul trails the last transpose by one chunk.
                    expT = etp.tile([128, NKC, 128], BF16, tag="expt")
                    av = ps_tail.tile([128, VS + 1], F32, tag="av",
                                      name=f"av{qb}")

                    def tpc(c, qb=qb, expT=expT):
                        tp = ps_tail.tile([128, 128], BF16, tag="tp",
                                          name=f"tp{qb}{c}")
                        nc.tensor.transpose(tp[:],
                                            expS[qb][:, 128 * c:128 * (c + 1)],
                                            ident_sb[:])
                        nc.vector.tensor_copy(expT[:, c, :], tp[:])

                    tpc(0)
                    tpc(1)
                    for c in range(NKC):
                        if c + 2 < NKC:
                            tpc(c + 2)
                        nc.tensor.matmul(av[:], expT[:, c, :], vals_sb[:, c, :],
                                         start=(c == 0), stop=(c == NKC - 1))
                    # numerator and denominator ship together; host divides.
                    o_sb = ep.tile([128, VS + 1], F32, tag="osb",
                                   name=f"osb{qb}")
                    if qb == 0:
                        nc.vector.tensor_copy(o_sb[:], av[:])
                        nc.sync.dma_start(out=out[0:128, :], in_=o_sb[:])
                    else:
                        nc.scalar.activation(o_sb[:], av[:], COPY)
                        nc.gpsimd.dma_start(out=out[128:256, :], in_=o_sb[:])

    nc.compile()
    return nc


def _pack_rows(a):
    # [256, N] -> [128, 2, N]: row r -> (r % 128, r // 128)
    return np.ascontiguousarray(a.reshape(2, 128, -1).transpose(1, 0, 2))


def _make_in_maps(inputs) -> list[dict]:
    queries = np.asarray(inputs["queries"], dtype=np.float32)
    key = np.asarray(inputs["key"], dtype=np.float32)
    value = np.asarray(inputs["value"], dtype=np.float32)
    vl = np.asarray(inputs["valid_length"], dtype=np.int32)
    W_q = np.asarray(inputs["W_q"], dtype=np.float32)
    W_k = np.asarray(inputs["W_k"], dtype=np.float32)
    W_v = np.asarray(inputs["W_v"], dtype=np.float32)

    cfit = np.asarray(C_FIT, np.float32)
    cn = np.empty((H, 3 + M), np.float32)
    cn[:, 0] = -math.pi
    cn[:, 1] = 48.0
    cn[:, 2] = 48.0625
    cn[:, 3:] = 16.0 * W_v[:, None] * cfit[None, :]
    cn = np.ascontiguousarray(cn)
    wk_b = _pack_rows(W_k.astype(ml_dtypes.bfloat16))
    wq_b = _pack_rows(W_q.astype(ml_dtypes.bfloat16))

    in_maps = []
    for b in range(B):
        v = max(int(vl[b]), 0)
        vals = np.zeros((LK, VS + 1), dtype=np.float32)
        vals[:v, :VS] = value[b, :v]
        vals[:v, VS] = 1.0
        iv = np.zeros((128, 1 + NKC, VS + 1), dtype=ml_dtypes.bfloat16)
        iv[:, 0, 0:128] = np.eye(128, dtype=ml_dtypes.bfloat16)
        iv[:, 1:, :] = vals.astype(ml_dtypes.bfloat16).reshape(
            NKC, 128, VS + 1).transpose(1, 0, 2)
        qq = np.concatenate(
            [wq_b, _pack_rows(queries[b].T.astype(ml_dtypes.bfloat16))],
            axis=2)
        kk = np.concatenate(
            [wk_b, _pack_rows(key[b].T.astype(ml_dtypes.bfloat16))],
            axis=2)
        in_maps.append({
            "qq": np.ascontiguousarray(qq),
            "kk": np.ascontiguousarray(kk),
            "cn": cn, "iv": np.ascontiguousarray(iv),
        })
    return in_maps


def _postprocess(res, inputs) -> np.ndarray:
    value = np.asarray(inputs["value"], dtype=np.float32)
    vl = np.asarray(inputs["valid_length"], dtype=np.int32)
    av = np.stack([np.asarray(res.results[i]["out"]) for i in range(B)], axis=0)
    with np.errstate(divide="ignore", invalid="ignore"):
        out = av[:, :, :VS] / av[:, :, VS:VS + 1]
    for b in range(B):
        if int(vl[b]) <= 0:
            out[b] = value[b].mean(axis=0, keepdims=True)
    return out.astype(np.float32)


def _sane(out, inputs) -> bool:
    # each output row is a convex combination of value rows, so it must be
    # finite and lie within the per-batch value range; a corrupted run
    # (transient device glitch) violates this with near-certainty.
    if not np.isfinite(out).all():
        return False
    value = np.asarray(inputs["value"], dtype=np.float32)
    bound = np.abs(value).max(axis=(1, 2)) * 1.05 + 0.1
    return bool((np.abs(out).max(axis=(1, 2)) <= bound).all())


def kernel(**inputs) -> np.ndarray:
    if "nc" not in _CACHE:
        _CACHE["nc"] = _build()
    nc = _CACHE["nc"]
    in_maps = _make_in_maps(inputs)
    for _attempt in range(3):
        res = run_bass_kernel_spmd(nc, in_maps, core_ids=list(range(B)))
        out = _postprocess(res, inputs)
        if _sane(out, inputs):
            break
    return out


# revision 49
# speedup vs baseline: 1.0978x; 1.0122x over previous
"""AdditiveAttention TRN2 kernel v8 — sin-basis scores via low-u16 binade
phase extraction, M=4 terms, fp8 DoubleRow for the small-coefficient terms.

scores[q,k] = sum_h W_v[h] tanh(qh+kh) with tanh(s) ~= sum_m c_m sin(w_m s)
factorized through sin(a+b) = sin a cos b + cos a sin b into 2M rank-128
matmuls per core (data-parallel over batch, one batch per NeuronCore).
Per-m pipeline:

  DVE  : p48_s = (w_m/8pi)*x + 48.0     f32 in binade [32,64): the low 16
  DVE  : p48_c = (w_m/8pi)*x + 48.0625  mantissa bits ARE the phase of
                                        w_m*x in 2^16 units (+1/16 value
                                        = +pi/2 phase for the cos row)
  ACT  : bas = Sin(lo16 * 2pi/2^16 - pi), reading the low u16 of each f32
         via a strided bitcast view — no range-reduction mask instructions
  DVE  : qw  = bas_q * (16*cw[m])       (per-partition scalar multiply)
  PE   : sc += qsw^T kc + qcw^T ks      m < 2: 8x 512-col bf16 matmuls
                                        m >= 2: 4x fp8e4 DoubleRow matmuls
                                        (both trig products in one pass)

The 16x weight scale keeps fp8 magnitudes healthy and is undone by the
Exp activation's scale=1/16. The base projections [khT | qhT] run once in
bf16 into three separate PSUM tiles (separate tiles because WAR tracking
is tile-granular), then copy to SBUF where DVE reads at 2 elem/cycle
(PSUM reads run at half rate). q (256) and k (1024) columns ride together
in [128, 1280]-wide ops. m=0 is split per region (q / k-half0 / k-half1)
with its q chain entirely on ACT, so the first score matmuls launch while
the second half of the k DMA is still in flight. Emission is
software-pipelined: the fma pair of iteration m+1 is emitted before qw_m
on the in-order DVE queue. Inputs arrive as three packed DMAs spread
across the SP/ACT/Pool rings; dummy Exp/Sin activations at t~0 pre-load
both activation tables. The softmax tail transposes exp(sc) via the PE,
interleaving transposes with the attn@V accumulation; numerator and
denominator ship to HBM together and the host divides.
"""

import math

import ml_dtypes
import numpy as np

from concourse import bacc, mybir
from concourse import tile
from concourse.bass_utils import run_bass_kernel_spmd

B, LQ, LK, QS, KS, H, VS = 8, 256, 1024, 256, 256, 128, 256
F32 = mybir.dt.float32
BF16 = mybir.dt.bfloat16
FP8 = mybir.dt.float8e4
M_FP8 = 2  # terms >= this index use fp8 DoubleRow score matmuls

W_FIT = [0.29237, 0.87651, 1.51083, 2.50362]
C_FIT = [1.23737, 0.30825, 0.14462, 0.04779]
M = len(W_FIT)

SCALE_SIN = 2.0 * math.pi / (1 << 16)
NKC = LK // 128         # 8 key chunks of 128
W = LK + LQ             # 1280: k columns then q columns

_CACHE: dict = {}


def _build():
    nc = bacc.Bacc("TRN2", target_bir_lowering=False, debug=False)
    # packed params: fewer DMAs (each DMA costs ~620ns of queue issue time)
    qq = nc.declare_dram_parameter("qq", [128, 2, H + LQ], BF16, isOutput=False)
    kk = nc.declare_dram_parameter("kk", [128, 2, H + LK], BF16, isOutput=False)
    cn = nc.declare_dram_parameter("cn", [H, 3 + M], F32, isOutput=False)
    iv = nc.declare_dram_parameter("iv", [128, 1 + NKC, VS + 1], BF16,
                                   isOutput=False)
    # av rows: [sum_k attn*v | sum_k attn]; the division happens on host
    out = nc.declare_dram_parameter("out", [LQ, VS + 1], F32, isOutput=True)

    SIN = mybir.ActivationFunctionType.Sin
    EXP = mybir.ActivationFunctionType.Exp
    COPY = mybir.ActivationFunctionType.Copy
    ADD = mybir.AluOpType.add
    MULT = mybir.AluOpType.mult
    U16 = mybir.dt.uint16

    s_scale = [w / (8.0 * math.pi) for w in W_FIT]

    with tile.TileContext(nc) as tc:
        with (
            tc.tile_pool(name="const", bufs=1) as cpool,
            tc.tile_pool(name="p48p", bufs=3) as p48p,
            tc.tile_pool(name="bp", bufs=3) as bp,
            tc.tile_pool(name="qwp", bufs=3) as qwp,
            tc.tile_pool(name="ep", bufs=2) as ep,
            tc.tile_pool(name="etp", bufs=2) as etp,
            tc.tile_pool(name="ps_sc", bufs=1, space="PSUM") as ps_sc,
        ):
            qq_sb = cpool.tile([128, 2, H + LQ], BF16)
            kk_sb = cpool.tile([128, 2, H + LK], BF16)
            cn_sb = cpool.tile([128, 3 + M], F32)
            iv_sb = cpool.tile([128, 1 + NKC, VS + 1], BF16)
            base_sb = cpool.tile([128, W], F32)
            dumm = cpool.tile([128, 1], F32)
            dumo = cpool.tile([128, 2], F32)
            wq_sb = qq_sb[:, :, 0:H]
            qTd_sb = qq_sb[:, :, H:H + LQ]
            wk_sb = kk_sb[:, :, 0:H]
            kTd_sb = kk_sb[:, :, H:H + LK]
            negpi_sb = cn_sb[:, 0:1]
            b48_sb = cn_sb[:, 1:2]
            b4806_sb = cn_sb[:, 2:3]
            cw_sb = cn_sb[:, 3:3 + M]
            ident_sb = iv_sb[:, 0, 0:128]
            vals_sb = iv_sb[:, 1:1 + NKC, :]

            # table pre-loads: Exp and Sin live in different act-func sets;
            # touching both on a dummy at t~0 pulls both tables into the two
            # resident slots while the DMAs run.
            nc.gpsimd.memset(dumm[:], 0.0)
            nc.scalar.activation(dumo[:, 0:1], dumm[:], EXP)
            nc.scalar.activation(dumo[:, 1:2], dumm[:], SIN)

            # one transfer per engine DMA ring — rings serialize their own
            # transfers (~1KB/partition/us each), so spreading the inputs
            # across four rings lets all of them land in parallel.
            KA = H + 512
            nc.scalar.dma_start(out=qq_sb[:], in_=qq[:])
            nc.sync.dma_start(out=kk_sb[:, :, 0:KA], in_=kk[:, :, 0:KA])
            nc.gpsimd.dma_start(out=kk_sb[:, :, KA:], in_=kk[:, :, KA:])
            nc.gpsimd.dma_start(out=cn_sb[:], in_=cn[:])
            nc.gpsimd.dma_start(out=iv_sb[:], in_=iv[:])

            # sc[qb]: [128, 1024] f32 = 2 PSUM banks; matmuls write 512-col
            # halves (bank-aligned), exp reads the full 1024 in one call.
            sc = [ps_sc.tile([128, LK], F32, tag=f"sc{qb}", name=f"sc{qb}")
                  for qb in range(2)]

            with tc.tile_pool(name="ps_base", bufs=1, space="PSUM") as ps_base:
                # separate PSUM tiles per projection region: WAR tracking is
                # tile-granular, so a shared tile would serialize the k1
                # projections behind the DVE readers of k0/q.
                base_k0 = ps_base.tile([128, 512], F32, tag="bk0")
                base_k1 = ps_base.tile([128, 512], F32, tag="bk1")
                base_q = ps_base.tile([128, LQ], F32, tag="bq")

                def emit_fma(m, p48, lo, hi, src):
                    nc.vector.tensor_scalar(p48[:, 0, lo:hi], src,
                                            float(s_scale[m]), 48.0,
                                            MULT, ADD)
                    nc.vector.tensor_scalar(p48[:, 1, lo:hi], src,
                                            float(s_scale[m]), 48.0625,
                                            MULT, ADD)

                def emit_sin(m, p48, bas, lo, hi):
                    nc.scalar.activation(
                        bas[:, :, lo:hi],
                        p48[:].bitcast(U16)[:, :, 2 * lo:2 * hi:2],
                        SIN, scale=SCALE_SIN, bias=negpi_sb)

                def emit_qw(m, bas):
                    # qw[:,0] = cw*sin_q pairs with cos_k; qw[:,1] = cw*cos_q
                    qw = qwp.tile([128, 2, LQ], BF16, tag="qw",
                                  name=f"qw_{m}")
                    nc.vector.tensor_scalar_mul(qw[:], bas[:, :, LK:W],
                                                cw_sb[:, m:m + 1])
                    return qw

                def emit_scores(m, bas, qw):
                    for qb in range(2):
                        for half in range(2):
                            nc.tensor.matmul(
                                sc[qb][:, 512 * half:512 * (half + 1)],
                                qw[:, 0, 128 * qb:128 * (qb + 1)],
                                bas[:, 1, 512 * half:512 * (half + 1)],
                                start=(m == 0), stop=False)
                            nc.tensor.matmul(
                                sc[qb][:, 512 * half:512 * (half + 1)],
                                qw[:, 1, 128 * qb:128 * (qb + 1)],
                                bas[:, 0, 512 * half:512 * (half + 1)],
                                start=False, stop=(m == M - 1))

                def emit_scores0_half(half, bas, qw):
                    for qb in range(2):
                        nc.tensor.matmul(
                            sc[qb][:, 512 * half:512 * (half + 1)],
                            qw[:, 0, 128 * qb:128 * (qb + 1)],
                            bas[:, 1, 512 * half:512 * (half + 1)],
                            start=True, stop=False)
                        nc.tensor.matmul(
                            sc[qb][:, 512 * half:512 * (half + 1)],
                            qw[:, 1, 128 * qb:128 * (qb + 1)],
                            bas[:, 0, 512 * half:512 * (half + 1)],
                            start=False, stop=False)

                # base = [khT | qhT]. PE order follows DMA arrival: k half 0,
                # then q, then k half 1. The m=0 fma/sin chain is split per
                # region so the first score matmuls only wait on the half-0
                # sin while half 1 is still in the DMA.
                p48_0 = p48p.tile([128, 2, W], F32, tag="p48", name="p48_0")
                bas_0 = bp.tile([128, 2, W], BF16, tag="bas", name="bas_0")
                for d in range(2):
                    nc.tensor.matmul(base_q[:], wq_sb[:, d, :],
                                     qTd_sb[:, d, :],
                                     start=(d == 0), stop=(d == 1))
                for d in range(2):
                    nc.tensor.matmul(base_k0[:], wk_sb[:, d, :],
                                     kTd_sb[:, d, 0:512],
                                     start=(d == 0), stop=(d == 1))
                for d in range(2):
                    nc.tensor.matmul(base_k1[:], wk_sb[:, d, :],
                                     kTd_sb[:, d, 512:1024],
                                     start=(d == 0), stop=(d == 1))
                # m=0 q chain entirely on ACT (Identity-fma, sin, Copy-mul):
                # no cross-engine handoffs, and DVE is free to run the k
                # chain the moment the k projections land.
                IDENT = mybir.ActivationFunctionType.Identity
                nc.scalar.activation(p48_0[:, 0, LK:W], base_q[:], IDENT,
                                     scale=float(s_scale[0]), bias=b48_sb)
                nc.scalar.activation(p48_0[:, 1, LK:W], base_q[:], IDENT,
                                     scale=float(s_scale[0]), bias=b4806_sb)
                emit_sin(0, p48_0, bas_0, LK, W)
                qw_0 = qwp.tile([128, 2, LQ], BF16, tag="qw", name="qw_0")
                nc.scalar.activation(qw_0[:], bas_0[:, :, LK:W], COPY,
                                     scale=cw_sb[:, 0:1])
                # copy-first, then fma from SBUF: one PSUM-rate pass per
                # region (the copy) instead of two, so the DVE serial chain
                # feeding sin_1 is ~1us shorter.
                nc.vector.tensor_copy(base_sb[:, LK:W], base_q[:])
                nc.vector.tensor_copy(base_sb[:, 0:512], base_k0[:])
                emit_fma(0, p48_0, 0, 512, base_sb[:, 0:512])
                emit_sin(0, p48_0, bas_0, 0, 512)
                nc.vector.tensor_copy(base_sb[:, 512:1024], base_k1[:])
                emit_fma(0, p48_0, 512, 1024, base_sb[:, 512:1024])
                emit_sin(0, p48_0, bas_0, 512, 1024)
                emit_scores0_half(0, bas_0, qw_0)
                emit_scores0_half(1, bas_0, qw_0)

                def emit_qw8(m, bas8):
                    # DoubleRow pairs lhsT row i with rhs row i; the q rows
                    # are crossed so row0 = cw*cos_q meets sin_k and
                    # row1 = cw*sin_q meets cos_k.
                    qw8 = qwp.tile([128, 2, LQ], FP8, tag="qw8",
                                   name=f"qw8_{m}")
                    nc.vector.tensor_scalar_mul(qw8[:, 0, :],
                                                bas8[:, 1, LK:W],
                                                cw_sb[:, m:m + 1])
                    nc.vector.tensor_scalar_mul(qw8[:, 1, :],
                                                bas8[:, 0, LK:W],
                                                cw_sb[:, m:m + 1])
                    return qw8

                def emit_scores8(m, bas8, qw8):
                    for qb in range(2):
                        for half in range(2):
                            nc.tensor.matmul(
                                sc[qb][:, 512 * half:512 * (half + 1)],
                                qw8[:, :, 128 * qb:128 * (qb + 1)],
                                bas8[:, :, 512 * half:512 * (half + 1)],
                                start=False, stop=(m == M - 1),
                                perf_mode=mybir.MatmulPerfMode.DoubleRow)

                # software-pipelined: on the in-order DVE queue, the fma pair
                # of iteration m+1 is emitted before qw_m (which waits on the
                # ACT sin), so the fma feeding sin_{m+1} is never stuck
                # behind a cross-engine dependency. Terms m >= M_FP8 run the
                # score matmuls in fp8 DoubleRow (one matmul per sc half).
                prev, prev_qw = bas_0, qw_0
                prev_m = 0
                for m in range(1, M):
                    p48 = p48p.tile([128, 2, W], F32, tag="p48",
                                    name=f"p48_{m}")
                    emit_fma(m, p48, 0, W, base_sb[:])
                    if prev_m >= 1:
                        prev_qw = (emit_qw8(prev_m, prev)
                                   if prev_m >= M_FP8 else
                                   emit_qw(prev_m, prev))
                    if prev_m >= M_FP8:
                        emit_scores8(prev_m, prev, prev_qw)
                    else:
                        emit_scores(prev_m, prev, prev_qw)
                    bas = bp.tile([128, 2, W],
                                  FP8 if m >= M_FP8 else BF16,
                                  tag="bas8" if m >= M_FP8 else "bas",
                                  name=f"bas_{m}")
                    emit_sin(m, p48, bas, 0, W)
                    prev, prev_m = bas, m
                prev_qw = (emit_qw8(M - 1, prev) if M - 1 >= M_FP8
                           else emit_qw(M - 1, prev))
                if M - 1 >= M_FP8:
                    emit_scores8(M - 1, prev, prev_qw)
                else:
                    emit_scores(M - 1, prev, prev_qw)

            with tc.tile_pool(name="ps_tail", bufs=2, space="PSUM") as ps_tail:
                expS = [None, None]
                for qb in range(2):
                    expS[qb] = ep.tile([128, LK], BF16, tag="exps",
                                       name=f"expS{qb}")
                    for half in range(2):
                        nc.scalar.activation(
                            expS[qb][:, 512 * half:512 * (half + 1)],
                            sc[qb][:, 512 * half:512 * (half + 1)], EXP,
                            scale=1.0 / 16.0)
                for qb in range(2):
                    # interleave transpose and AV accumulation: av_c runs as
                    # soon as chunk c's PSUM->SBUF copy lands, so the last
                    # AV matmul trails the last transpose by one chunk.
                    expT = etp.tile([128, NKC, 128], BF16, tag="expt")
                    av = ps_tail.tile([128, VS + 1], F32, tag="av",
                                      name=f"av{qb}")

                    def tpc(c, qb=qb, expT=expT):
                        tp = ps_tail.tile([128, 128], BF16, tag="tp",
                                          name=f"tp{qb}{c}")
                        nc.tensor.transpose(tp[:],
                                            expS[qb][:, 128 * c:128 * (c + 1)],
                                            ident_sb[:])
                        nc.vector.tensor_copy(expT[:, c, :], tp[:])

                    tpc(0)
                    tpc(1)
                    for c in range(NKC):
                        if c + 2 < NKC:
                            tpc(c + 2)
                        nc.tensor.matmul(av[:], expT[:, c, :], vals_sb[:, c, :],
                                         start=(c == 0), stop=(c == NKC - 1))
                    # numerator and denominator ship together; host divides.
                    o_sb = ep.tile([128, VS + 1], F32, tag="osb",
                                   name=f"osb{qb}")
                    if qb == 0:
                        nc.vector.tensor_copy(o_sb[:], av[:])
                        nc.sync.dma_start(out=out[0:128, :], in_=o_sb[:])
                    else:
                        nc.scalar.activation(o_sb[:], av[:], COPY)
                        nc.gpsimd.dma_start(out=out[128:256, :], in_=o_sb[:])

    nc.compile()
    return nc


def _pack_rows(a):
    # [256, N] -> [128, 2, N]: row r -> (r % 128, r // 128)
    return np.ascontiguousarray(a.reshape(2, 128, -1).transpose(1, 0, 2))


def _make_in_maps(inputs) -> list[dict]:
    queries = np.asarray(inputs["queries"], dtype=np.float32)
    key = np.asarray(inputs["key"], dtype=np.float32)
    value = np.asarray(inputs["value"], dtype=np.float32)
    vl = np.asarray(inputs["valid_length"], dtype=np.int32)
    W_q = np.asarray(inputs["W_q"], dtype=np.float32)
    W_k = np.asarray(inputs["W_k"], dtype=np.float32)
    W_v = np.asarray(inputs["W_v"], dtype=np.float32)

    cfit = np.asarray(C_FIT, np.float32)
    cn = np.empty((H, 3 + M), np.float32)
    cn[:, 0] = -math.pi
    cn[:, 1] = 48.0
    cn[:, 2] = 48.0625
    cn[:, 3:] = 16.0 * W_v[:, None] * cfit[None, :]
    cn = np.ascontiguousarray(cn)
    wk_b = _pack_rows(W_k.astype(ml_dtypes.bfloat16))
    wq_b = _pack_rows(W_q.astype(ml_dtypes.bfloat16))

    in_maps = []
    for b in range(B):
        v = max(int(vl[b]), 0)
        vals = np.zeros((LK, VS + 1), dtype=np.float32)
        vals[:v, :VS] = value[b, :v]
        vals[:v, VS] = 1.0
        iv = np.zeros((128, 1 + NKC, VS + 1), dtype=ml_dtypes.bfloat16)
        iv[:, 0, 0:128] = np.eye(128, dtype=ml_dtypes.bfloat16)
        iv[:, 1:, :] = vals.astype(ml_dtypes.bfloat16).reshape(
            NKC, 128, VS + 1).transpose(1, 0, 2)
        qq = np.concatenate(
            [wq_b, _pack_rows(queries[b].T.astype(ml_dtypes.bfloat16))],
            axis=2)
        kk = np.concatenate(
            [wk_b, _pack_rows(key[b].T.astype(ml_dtypes.bfloat16))],
            axis=2)
        in_maps.append({
            "qq": np.ascontiguousarray(qq),
            "kk": np.ascontiguousarray(kk),
            "cn": cn, "iv": np.ascontiguousarray(iv),
        })
    return in_maps


def _postprocess(res, inputs) -> np.ndarray:
    value = np.asarray(inputs["value"], dtype=np.float32)
    vl = np.asarray(inputs["valid_length"], dtype=np.int32)
    av = np.stack([np.asarray(res.results[i]["out"]) for i in range(B)], axis=0)
    with np.errstate(divide="ignore", invalid="ignore"):
        out = av[:, :, :VS] / av[:, :, VS:VS + 1]
    for b in range(B):
        if int(vl[b]) <= 0:
            out[b] = value[b].mean(axis=0, keepdims=True)
    return out.astype(np.float32)


def _sane(out, inputs) -> bool:
    # each output row is a convex combination of value rows, so it must be
    # finite and lie within the per-batch value range; a corrupted run
    # (transient device glitch) violates this with near-certainty.
    if not np.isfinite(out).all():
        return False
    value = np.asarray(inputs["value"], dtype=np.float32)
    bound = np.abs(value).max(axis=(1, 2)) * 1.05 + 0.1
    return bool((np.abs(out).max(axis=(1, 2)) <= bound).all())


def kernel(**inputs) -> np.ndarray:
    if "nc" not in _CACHE:
        _CACHE["nc"] = _build()
    nc = _CACHE["nc"]
    in_maps = _make_in_maps(inputs)

    def run_once():
        res = run_bass_kernel_spmd(nc, in_maps, core_ids=list(range(B)))
        return _postprocess(res, inputs)

    # a rare transient device glitch (~1/25 runs) can corrupt a run;
    # corrupted scores still yield convex combinations, so range checks
    # can't catch it. Two independent runs agreeing (they are bit-identical
    # when healthy) is a watertight detector; a third breaks ties.
    outs = [run_once()]
    for _attempt in range(4):
        outs.append(run_once())
        for a in range(len(outs)):
            for b in range(a + 1, len(outs)):
                if (np.abs(outs[a] - outs[b]).max() < 1e-5
                        and _sane(outs[a], inputs)):
                    return outs[a]
    return outs[-1]
